# revision 4
# baseline (speedup 1.0000x reference)
"""Trainium2 Bass kernel for a causal self-attention transformer layer.

Layer (PostNorm, eval):
    h  = MHA_causal(tgt); x = LN(tgt + h); out = LN(x + gelu(x@W1.T+b1)@W2.T+b2)
Shapes: B=2, N=2048, D=1024, H=16 (dk=64), FFN=4096.

Distribution over 8 NeuronCores (core g: batch b=g//4, rank r=g%4):
  Stage A (head-parallel): core computes Q/K/V + causal attention for its 4
    heads over all 2048 tokens of its batch. Activations are kept
    feature-major [feat, tok] so every matmul contracts on partitions.
    Softmax uses no max-subtraction (scores are O(1) here); the denominator
    is produced for free as an extra all-ones column of V in the AV matmul.
  AllToAll (8 cores) converts head-sharded attention output into
    token-sharded (512 tokens/core). Cross-batch slots carry duplicate data
    which is nulled by zero-padded Wo weights.
  Stage B (token-parallel): Wo + bias + residual, LN1, W1+gelu, W2 + bias +
    residual, LN2 for the core's 512 tokens. LayerNorm over the feature
    (partition) axis uses ones-vector matmuls for the sums and K=1
    broadcast matmuls (float32r) to spread the per-token mean/rstd.

Host side shards/transposes/casts inputs, runs the SPMD program via a
cached PJRT callable, and reassembles the full [2, 2048, 1024] output.
"""

import numpy as np
import ml_dtypes

import concourse.bass as bass
import concourse.mybir as mybir
import concourse.tile as tile
from concourse.vector_clock import ScopedClock

BF16 = mybir.dt.bfloat16
F32 = mybir.dt.float32
F32R = mybir.dt.float32r
AF = mybir.ActivationFunctionType
ALU = mybir.AluOpType

B, N, D, H, DK, FFN = 2, 2048, 1024, 16, 64, 4096
EPS = 1e-5
NCORES = 8
TPR = 4            # tensor-parallel ranks per batch
HL = H // TPR      # heads per core (4)
DVL = HL * DK      # local head width (256)
TS = N // TPR      # tokens per core in stage B (512)
NP = 128           # partitions
QT = 512           # q tile width
NKB = N // NP      # key blocks (16)

# ---------------------------------------------------------------------------
# Walrus in this environment encodes at most ONE sync-wait per instruction.
# Patch Tile's exit drain and post-split every multi-wait instruction.
# ---------------------------------------------------------------------------

_wsplit = [0]


def _patched_drain_and_barrier(self, tick_clock, wait_clock):
    nc = self.nc
    probe = nc.sync.nop(nofuse=True)
    wait_clock.add_sem_waits(probe.ins, ScopedClock({None: tick_clock.global_clock}))
    si = probe.ins.sync_info
    waits = list(si.on_wait) if si is not None else []
    if waits:
        probe.ins.sync_info = mybir.SyncInfo(on_wait=[waits[0]], on_update=[])
        for w in waits[1:]:
            extra = nc.sync.nop(nofuse=True)
            extra.ins.sync_info = mybir.SyncInfo(on_wait=[w], on_update=[])
    nc.sync.drain()
    nc.all_engine_barrier()
    popped = nc._tile_sem_poison_stack.pop()
    assert popped is self._sem_poison
    nc.clear_and_free_semaphores(list(self.sems.allocated().values()))
    nc.all_engine_barrier()


tile.TileContext._drain_and_barrier = _patched_drain_and_barrier


def _split_multiwait_instructions(nc):
    for fn in nc.m.functions:
        for bb in fn.blocks:
            insts = bb.instructions
            if not any(
                i.sync_info is not None and len(i.sync_info.on_wait) > 1
                for i in insts
            ):
                continue
            new = []
            for inst in insts:
                si = inst.sync_info
                waits = list(si.on_wait) if si is not None else []
                if len(waits) > 1:
                    for w in waits[:-1]:
                        _wsplit[0] += 1
                        new.append(mybir.InstNoOp(
                            name=f"wsplit-{_wsplit[0]}",
                            engine=inst.engine,
                            sync_info=mybir.SyncInfo(on_wait=[w], on_update=[]),
                        ))
                    inst.sync_info = mybir.SyncInfo(
                        on_wait=[waits[-1]], on_update=list(si.on_update)
                    )
                new.append(inst)
            bb.instructions = new


# ---------------------------------------------------------------------------
# Program builder
# ---------------------------------------------------------------------------

def _build_program():
    nc = bass.Bass("TRN2", target_bir_lowering=False, debug=False,
                   num_devices=NCORES)

    def din(name, shape, dt):
        return nc.dram_tensor(name, shape, dt, kind="ExternalInput").ap()

    xt_d = din("xt", [D, N], BF16)            # tgt[b].T
    xtr_d = din("xtr", [D, TS], F32)          # residual slice (my tokens)
    wqt_d = din("wqt", [D, DVL], BF16)        # Wq[local].T
    wkt_d = din("wkt", [D, DVL], BF16)
    wvt_d = din("wvt", [D, DVL], BF16)
    wot_d = din("wot", [2 * D, D], BF16)      # zero-padded Wo.T (A2A slots)
    w1t_d = din("w1t", [D, FFN], BF16)
    w2t_d = din("w2t", [FFN, D], BF16)
    bo_d = din("bo", [NP, D // NP], F32)
    b1_d = din("b1", [NP, FFN // NP], F32)
    b2_d = din("b2", [NP, D // NP], F32)
    mask_d = din("mask2", [NP, 4, 2 * QT], BF16)
    out_d = nc.dram_tensor("out", [D, TS], F32, kind="ExternalOutput").ap()

    cc_in = nc.dram_tensor("cc_in", [NCORES * DVL, TS], BF16).ap()
    cc_out = nc.dram_tensor("cc_out", [NCORES * DVL, TS], BF16).ap()

    with tile.TileContext(nc, num_cores=NCORES) as tc:
        with tc.tile_pool(name="const", bufs=1) as cpool:
            onesf = cpool.tile([NP, NP], F32)
            nc.vector.memset(onesf[:], 1.0)
            ones64r = cpool.tile([NP, 64], F32R)   # row 64 used by AV bcast
            nc.vector.tensor_copy(ones64r[64:65, :], onesf[64:65, 0:64])
            ones128r = cpool.tile([1, NP], F32R)   # LN bcast lhsT
            nc.vector.tensor_copy(ones128r[:], onesf[0:1, :])
            ones128b = cpool.tile([NP, 1], BF16)   # LN stats lhsT
            nc.vector.memset(ones128b[:], 1.0)
            bo_sb = cpool.tile([NP, D // NP], F32)
            nc.sync.dma_start(out=bo_sb[:], in_=bo_d[:])
            b1_sb = cpool.tile([NP, FFN // NP], F32)
            nc.sync.dma_start(out=b1_sb[:], in_=b1_d[:])
            b2_sb = cpool.tile([NP, D // NP], F32)
            nc.sync.dma_start(out=b2_sb[:], in_=b2_d[:])
            mask_sb = cpool.tile([NP, 4, 2 * QT], BF16)
            nc.sync.dma_start(out=mask_sb[:], in_=mask_d[:])

            # ---------------- Stage A: QKV + attention (4 local heads) ----
            with tc.tile_pool(name="sa", bufs=1) as sa:
                xt_all = sa.tile([NP, D // NP, N], BF16)
                nc.sync.dma_start(
                    out=xt_all[:], in_=xt_d.rearrange("(c p) t -> p c t", p=NP))
                wq_sb = sa.tile([NP, D // NP, DVL], BF16)
                nc.sync.dma_start(
                    out=wq_sb[:], in_=wqt_d.rearrange("(c p) f -> p c f", p=NP))
                wk_sb = sa.tile([NP, D // NP, DVL], BF16)
                nc.sync.dma_start(
                    out=wk_sb[:], in_=wkt_d.rearrange("(c p) f -> p c f", p=NP))
                wv_sb = sa.tile([NP, D // NP, DVL], BF16)
                nc.sync.dma_start(
                    out=wv_sb[:], in_=wvt_d.rearrange("(c p) f -> p c f", p=NP))

                q_sb = sa.tile([NP, 2, N], BF16)
                k_sb = sa.tile([NP, 2, N], BF16)
                v_sb = sa.tile([NP, NKB, HL * (DK + 1)], BF16)

                with tc.tile_pool(name="qkv_ps", bufs=1, space="PSUM") as qp:
                    for o in range(2):
                        for t in range(N // QT):
                            ps = qp.tile([NP, QT], F32, tag="qk", bufs=3)
                            for dc in range(D // NP):
                                nc.tensor.matmul(
                                    ps[:],
                                    wq_sb[:, dc, o * NP:(o + 1) * NP],
                                    xt_all[:, dc, t * QT:(t + 1) * QT],
                                    start=(dc == 0), stop=(dc == D // NP - 1))
                            nc.scalar.activation(
                                q_sb[:, o, t * QT:(t + 1) * QT], ps[:], AF.Copy)
                            ps2 = qp.tile([NP, QT], F32, tag="qk", bufs=3)
                            for dc in range(D // NP):
                                nc.tensor.matmul(
                                    ps2[:],
                                    wk_sb[:, dc, o * NP:(o + 1) * NP],
                                    xt_all[:, dc, t * QT:(t + 1) * QT],
                                    start=(dc == 0), stop=(dc == D // NP - 1))
                            nc.vector.tensor_copy(
                                k_sb[:, o, t * QT:(t + 1) * QT], ps2[:])
                    for t in range(NKB):
                        psv = qp.tile([NP, DVL], F32, tag="v", bufs=2)
                        for dc in range(D // NP):
                            nc.tensor.matmul(
                                psv[:],
                                xt_all[:, dc, t * NP:(t + 1) * NP],
                                wv_sb[:, dc, :],
                                start=(dc == 0), stop=(dc == D // NP - 1))
                        vview = v_sb[:, t, :].rearrange("p (h c) -> p h c", c=DK + 1)
                        nc.vector.tensor_copy(
                            vview[:, :, 0:DK],
                            psv[:].rearrange("p (h c) -> p h c", c=DK))
                        nc.vector.memset(vview[:, :, DK:DK + 1], 1.0)

                # attention: per q-tile j, per head-pair hp
                attn_h = [sa.tile([DK, N], BF16, tag=f"attn{h}", name=f"attn{h}")
                          for h in range(HL)]
                with tc.tile_pool(name="att_ps", bufs=1, space="PSUM") as ap:
                    for j in range(N // QT):
                        for hp in range(2):
                            h0, h1 = 2 * hp, 2 * hp + 1
                            pav0 = ap.tile([NP, QT], F32, tag="av0")
                            pav1 = ap.tile([NP, QT], F32, tag="av1")
                            nkb = (j + 1) * (QT // NP)
                            for kb in range(nkb):
                                ps_s = ap.tile([NP, 2 * QT], F32, tag="s", bufs=2)
                                nc.tensor.matmul(
                                    ps_s[:, 0:QT],
                                    k_sb[0:64, hp, kb * NP:(kb + 1) * NP],
                                    q_sb[0:64, hp, j * QT:(j + 1) * QT],
                                    start=True, stop=True)
                                nc.tensor.matmul(
                                    ps_s[:, QT:2 * QT],
                                    k_sb[64:NP, hp, kb * NP:(kb + 1) * NP],
                                    q_sb[64:NP, hp, j * QT:(j + 1) * QT],
                                    start=True, stop=True)
                                e_sb = sa.tile([NP, 2 * QT], BF16, tag="e",
                                               bufs=3)
                                nc.scalar.activation(
                                    e_sb[:], ps_s[:], AF.Exp,
                                    scale=1.0 / np.sqrt(DK))
                                di = kb - (QT // NP) * j
                                if di >= 0:
                                    nc.vector.tensor_tensor(
                                        e_sb[:], e_sb[:], mask_sb[:, di, :],
                                        op=ALU.mult)
                                nc.tensor.matmul(
                                    pav0[0:DK + 1, :],
                                    v_sb[:, kb, h0 * (DK + 1):(h0 + 1) * (DK + 1)],
                                    e_sb[:, 0:QT],
                                    start=(kb == 0), stop=(kb == nkb - 1))
                                nc.tensor.matmul(
                                    pav1[0:DK + 1, :],
                                    v_sb[:, kb, h1 * (DK + 1):(h1 + 1) * (DK + 1)],
                                    e_sb[:, QT:2 * QT],
                                    start=(kb == 0), stop=(kb == nkb - 1))
                            for e01, pav, h in ((0, pav0, h0), (1, pav1, h1)):
                                rr = sa.tile([NP, QT], F32, tag="rr", bufs=2)
                                nc.vector.reciprocal(
                                    rr[64:65, :], pav[DK:DK + 1, :])
                                rrr = sa.tile([NP, QT], F32R, tag="rrr", bufs=2)
                                nc.vector.tensor_copy(rrr[64:65, :], rr[64:65, :])
                                pb = ap.tile([DK, QT], F32, tag="pb", bufs=2)
                                nc.tensor.matmul(
                                    pb[:], ones64r[64:65, :], rrr[64:65, :],
                                    start=True, stop=True)
                                rbc = sa.tile([DK, QT], F32, tag="rbc", bufs=2)
                                nc.scalar.activation(rbc[:], pb[:], AF.Copy)
                                nc.vector.tensor_tensor(
                                    attn_h[h][:, j * QT:(j + 1) * QT],
                                    pav[0:DK, :], rbc[:], op=ALU.mult)
                        # stage chunk j of all 4 heads into cc_in slots j, j+4
                        for h in range(HL):
                            for s in (j, j + TPR):
                                nc.sync.dma_start(
                                    out=cc_in[DVL * s + DK * h:
                                              DVL * s + DK * (h + 1), :],
                                    in_=attn_h[h][:, j * QT:(j + 1) * QT])

            nc.gpsimd.collective_compute(
                "AllToAll", ALU.bypass,
                ins=[cc_in[:]], outs=[cc_out[:]],
                replica_groups=[list(range(NCORES))],
            )

            # ---------------- Stage B: Wo + LN1 + MLP + LN2 (512 tokens) --
            def layer_norm(tc, pool, src, outf, outb):
                """src/outf [NP, 8, TS] f32; outb [NP, 8, TS] bf16 or None."""
                nblk = D // NP
                sbf = pool.tile([NP, nblk, TS], BF16, tag="ln_bf")
                sqb = pool.tile([NP, nblk, TS], BF16, tag="ln_sq")
                with tc.tile_pool(name="ln_ps", bufs=1, space="PSUM") as lp:
                    pmu = lp.tile([1, TS], F32, tag="mu")
                    psq = lp.tile([1, TS], F32, tag="sq")
                    for ob in range(nblk):
                        nc.vector.tensor_copy(sbf[:, ob, :], src[:, ob, :])
                        nc.vector.tensor_tensor(
                            sqb[:, ob, :], sbf[:, ob, :], sbf[:, ob, :],
                            op=ALU.mult)
                        nc.tensor.matmul(pmu[:], ones128b[:], sbf[:, ob, :],
                                         start=(ob == 0), stop=(ob == nblk - 1))
                        nc.tensor.matmul(psq[:], ones128b[:], sqb[:, ob, :],
                                         start=(ob == 0), stop=(ob == nblk - 1))
                    rows = pool.tile([1, 7, TS], F32, tag="ln_rows")
                    mu, ex2, mu2, var = (rows[:, i, :] for i in range(4))
                    vr, rstd, brow = (rows[:, i, :] for i in range(4, 7))
                    nc.vector.tensor_scalar_mul(mu, pmu[:], 1.0 / D)
                    nc.vector.tensor_scalar_mul(ex2, psq[:], 1.0 / D)
                    nc.vector.tensor_tensor(mu2, mu, mu, op=ALU.mult)
                    nc.vector.tensor_tensor(var, ex2, mu2, op=ALU.subtract)
                    nc.vector.tensor_scalar_add(var, var, EPS)
                    nc.vector.reciprocal(vr, var)
                    nc.scalar.activation(rstd, vr, AF.Sqrt)
                    nc.vector.tensor_tensor(brow, mu, rstd, op=ALU.mult)
                    rowr = pool.tile([1, 2, TS], F32R, tag="ln_rowr")
                    nc.vector.tensor_copy(rowr[:, 0, :], rstd)
                    nc.vector.tensor_scalar_mul(rowr[:, 1, :], brow, -1.0)
                    pA = lp.tile([NP, TS], F32, tag="bA")
                    pB = lp.tile([NP, TS], F32, tag="bB")
                    nc.tensor.matmul(pA[:], ones128r[:], rowr[:, 0, :],
                                     start=True, stop=True)
                    nc.tensor.matmul(pB[:], ones128r[:], rowr[:, 1, :],
                                     start=True, stop=True)
                    A_sb = pool.tile([NP, TS], F32, tag="ln_A")
                    B_sb = pool.tile([NP, TS], F32, tag="ln_B")
                    nc.scalar.activation(A_sb[:], pA[:], AF.Copy)
                    nc.scalar.activation(B_sb[:], pB[:], AF.Copy)
                    tmp = pool.tile([NP, nblk, TS], F32, tag="ln_tmp")
                    for ob in range(nblk):
                        nc.vector.tensor_tensor(
                            tmp[:, ob, :], src[:, ob, :], A_sb[:], op=ALU.mult)
                        nc.vector.tensor_tensor(
                            outf[:, ob, :], tmp[:, ob, :], B_sb[:], op=ALU.add)
                        if outb is not None:
                            nc.vector.tensor_copy(outb[:, ob, :], outf[:, ob, :])

            with tc.tile_pool(name="sb1", bufs=1) as sb1:
                sum1 = sb1.tile([NP, D // NP, TS], F32)
                with tc.tile_pool(name="sbo", bufs=1) as sbo, \
                     tc.tile_pool(name="wo_ps", bufs=1, space="PSUM") as wp:
                    ao = sbo.tile([NP, 2 * D // NP, TS], BF16)
                    nc.sync.dma_start(
                        out=ao[:], in_=cc_out.rearrange("(c p) t -> p c t", p=NP))
                    wot_sb = sbo.tile([NP, 2 * D // NP, D], BF16)
                    nc.sync.dma_start(
                        out=wot_sb[:],
                        in_=wot_d.rearrange("(c p) f -> p c f", p=NP))
                    xtr_sb = sbo.tile([NP, D // NP, TS], F32)
                    nc.sync.dma_start(
                        out=xtr_sb[:],
                        in_=xtr_d.rearrange("(c p) t -> p c t", p=NP))
                    for ob in range(D // NP):
                        ph = wp.tile([NP, TS], F32, tag="h", bufs=2)
                        for c in range(2 * D // NP):
                            nc.tensor.matmul(
                                ph[:], wot_sb[:, c, ob * NP:(ob + 1) * NP],
                                ao[:, c, :],
                                start=(c == 0), stop=(c == 2 * D // NP - 1))
                        hb = sbo.tile([NP, TS], F32, tag="hb", bufs=2)
                        nc.scalar.activation(hb[:], ph[:], AF.Identity,
                                             bias=bo_sb[:, ob:ob + 1])
                        nc.vector.tensor_tensor(
                            sum1[:, ob, :], hb[:], xtr_sb[:, ob, :], op=ALU.add)

                x2f = sb1.tile([NP, D // NP, TS], F32)
                x2b = sb1.tile([NP, D // NP, TS], BF16)
                with tc.tile_pool(name="ln1", bufs=1) as lnp:
                    layer_norm(tc, lnp, sum1, x2f, x2b)

                g_all = sb1.tile([NP, FFN // NP, TS], BF16)
                with tc.tile_pool(name="w1s", bufs=1) as w1s, \
                     tc.tile_pool(name="w1_ps", bufs=1, space="PSUM") as mp:
                    for fc in range(FFN // QT):
                        w1c = w1s.tile([NP, D // NP, QT], BF16, tag="w1c",
                                       bufs=2)
                        nc.sync.dma_start(
                            out=w1c[:],
                            in_=w1t_d[:, fc * QT:(fc + 1) * QT].rearrange(
                                "(c p) f -> p c f", p=NP))
                        for fs in range(QT // NP):
                            fb = fc * (QT // NP) + fs
                            pm = mp.tile([NP, TS], F32, tag="m", bufs=3)
                            for dc in range(D // NP):
                                nc.tensor.matmul(
                                    pm[:], w1c[:, dc, fs * NP:(fs + 1) * NP],
                                    x2b[:, dc, :],
                                    start=(dc == 0), stop=(dc == D // NP - 1))
                            nc.scalar.activation(
                                g_all[:, fb, :], pm[:], AF.Gelu_apprx_tanh,
                                bias=b1_sb[:, fb:fb + 1])

                sum2 = sb1.tile([NP, D // NP, TS], F32)
                with tc.tile_pool(name="w2s", bufs=1) as w2s, \
                     tc.tile_pool(name="w2_ps", bufs=1, space="PSUM") as yp:
                    pys = [yp.tile([NP, TS], F32, tag=f"y{ob}", name=f"y{ob}")
                           for ob in range(D // NP)]
                    for fb in range(FFN // NP):
                        w2c = w2s.tile([NP, D], BF16, tag="w2c", bufs=3)
                        nc.sync.dma_start(
                            out=w2c[:], in_=w2t_d[fb * NP:(fb + 1) * NP, :])
                        for ob in range(D // NP):
                            nc.tensor.matmul(
                                pys[ob][:], w2c[:, ob * NP:(ob + 1) * NP],
                                g_all[:, fb, :],
                                start=(fb == 0), stop=(fb == FFN // NP - 1))
                    for ob in range(D // NP):
                        mb = w2s.tile([NP, TS], F32, tag="mb", bufs=2)
                        nc.scalar.activation(mb[:], pys[ob][:], AF.Identity,
                                             bias=b2_sb[:, ob:ob + 1])
                        nc.vector.tensor_tensor(
                            sum2[:, ob, :], mb[:], x2f[:, ob, :], op=ALU.add)

                yf = sb1.tile([NP, D // NP, TS], F32)
                with tc.tile_pool(name="ln2", bufs=1) as lnp2:
                    layer_norm(tc, lnp2, sum2, yf, None)
                for ob in range(D // NP):
                    nc.sync.dma_start(
                        out=out_d[ob * NP:(ob + 1) * NP, :], in_=yf[:, ob, :])

    _split_multiwait_instructions(nc)
    return nc


# ---------------------------------------------------------------------------
# Cached PJRT runner (mirrors bass2jax.run_bass_via_pjrt multi-core path but
# keeps the jitted callable so repeat calls don't recompile).
# ---------------------------------------------------------------------------

_RUNNER = None


def _make_runner(nc):
    import jax
    from jax.sharding import Mesh, PartitionSpec
    from jax.experimental.shard_map import shard_map
    from concourse import bass2jax

    bass2jax.install_neuronx_cc_hook()
    partition_name = (nc.partition_id_tensor.name
                      if nc.partition_id_tensor else None)
    in_names, out_names, out_avals = [], [], []
    for alloc in nc.m.functions[0].allocations:
        if not isinstance(alloc, mybir.MemoryLocationSet):
            continue
        name = alloc.memorylocations[0].name
        if alloc.kind == "ExternalInput":
            if name != partition_name:
                in_names.append(name)
        elif alloc.kind == "ExternalOutput":
            out_names.append(name)
            out_avals.append(jax.core.ShapedArray(
                tuple(alloc.tensor_shape), mybir.dt.np(alloc.dtype)))
    n_params = len(in_names)
    all_in_names = list(in_names) + list(out_names)
    if partition_name is not None:
        all_in_names.append(partition_name)
    donate = tuple(range(n_params, n_params + len(out_names)))

    def _body(*args):
        operands = list(args)
        if partition_name is not None:
            operands.append(bass2jax.partition_id_tensor())
        outs = bass2jax._bass_exec_p.bind(
            *operands,
            out_avals=tuple(out_avals),
            in_names=tuple(all_in_names),
            out_names=tuple(out_names),
            lowering_input_output_aliases=(),
            sim_require_finite=True,
            sim_require_nnan=True,
            nc=nc,
        )
        return tuple(outs)

    devices = jax.devices()[:NCORES]
    mesh = Mesh(np.asarray(devices), ("core",))
    specs = (PartitionSpec("core"),) * (n_params + len(out_names))
    sharded = jax.jit(
        shard_map(_body, mesh=mesh, in_specs=specs,
                  out_specs=(PartitionSpec("core"),) * len(out_names),
                  check_rep=False),
        donate_argnums=donate, keep_unused=True)

    def run(in_maps, materialize=True):
        per_core = [[np.asarray(m[name]) for name in in_names]
                    for m in in_maps]
        concat_in = [np.concatenate([per_core[c][i] for c in range(NCORES)],
                                    axis=0) for i in range(n_params)]
        concat_zeros = [np.zeros((NCORES * a.shape[0], *a.shape[1:]), a.dtype)
                        for a in out_avals]
        out_arrs = sharded(*concat_in, *concat_zeros)
        if not materialize:
            jax.block_until_ready(out_arrs)
            return None
        return [
            {name: np.asarray(out_arrs[i]).reshape(
                NCORES, *out_avals[i].shape)[c]
             for i, name in enumerate(out_names)}
            for c in range(NCORES)
        ]

    return run


def _get_runner():
    global _RUNNER
    if _RUNNER is None:
        nc = _build_program()
        _RUNNER = _make_runner(nc)
    return _RUNNER


# ---------------------------------------------------------------------------
# Host-side sharding / gathering
# ---------------------------------------------------------------------------

def _bf16(a):
    return np.ascontiguousarray(a).astype(ml_dtypes.bfloat16)


def make_in_maps(tgt, tgt_mask, tgt_key_pad_mask, Wq, Wk, Wv, Wo, bo,
                 W1, b1, W2, b2, g1, beta1, g2, beta2):
    causal = np.triu(np.ones((N, N), bool), k=1)
    if not (np.array_equal(np.asarray(tgt_mask), causal)
            and not np.asarray(tgt_key_pad_mask).any()
            and np.allclose(np.asarray(g1), 1) and np.allclose(np.asarray(g2), 1)
            and np.allclose(np.asarray(beta1), 0)
            and np.allclose(np.asarray(beta2), 0)):
        return None  # unsupported masking/affine -> numpy fallback

    tgt = np.asarray(tgt, np.float32)
    Wq, Wk, Wv, Wo = (np.asarray(a, np.float32) for a in (Wq, Wk, Wv, Wo))
    W1, W2 = np.asarray(W1, np.float32), np.asarray(W2, np.float32)
    bo, b1, b2 = (np.asarray(a, np.float32) for a in (bo, b1, b2))

    # diagonal-block multiplicative masks [128, 4, 1024]
    q_idx = np.arange(QT)[None, :]
    mask2 = np.empty((NP, 4, 2 * QT), np.float32)
    for i in range(4):
        m = (NP * i + np.arange(NP)[:, None] <= q_idx).astype(np.float32)
        mask2[:, i, :] = np.concatenate([m, m], axis=1)

    w1t = _bf16(W1.T)
    w2t = _bf16(W2.T)
    bo_c = np.ascontiguousarray(bo.reshape(D // NP, NP).T)
    b1_c = np.ascontiguousarray(b1.reshape(FFN // NP, NP).T)
    b2_c = np.ascontiguousarray(b2.reshape(D // NP, NP).T)
    mask2_bf = _bf16(mask2)

    xt_b = [_bf16(tgt[b].T) for b in range(B)]
    in_maps = []
    for g in range(NCORES):
        b, r = g // TPR, g % TPR
        sl = slice(DVL * r, DVL * (r + 1))
        wot_ext = np.zeros((2 * D, D), np.float32)
        wot_ext[D * b:D * (b + 1), :] = Wo.T
        in_maps.append({
            "xt": xt_b[b],
            "xtr": np.ascontiguousarray(tgt[b].T[:, TS * r:TS * (r + 1)]),
            "wqt": _bf16(Wq[sl, :].T),
            "wkt": _bf16(Wk[sl, :].T),
            "wvt": _bf16(Wv[sl, :].T),
            "wot": _bf16(wot_ext),
            "w1t": w1t,
            "w2t": w2t,
            "bo": bo_c,
            "b1": b1_c,
            "b2": b2_c,
            "mask2": mask2_bf,
        })
    return in_maps


def _numpy_reference(tgt, tgt_mask, tgt_key_pad_mask, Wq, Wk, Wv, Wo, bo,
                     W1, b1, W2, b2, g1, beta1, g2, beta2):
    def ln(x, g, b):
        mu = x.mean(-1, keepdims=True)
        var = ((x - mu) ** 2).mean(-1, keepdims=True)
        return (x - mu) / np.sqrt(var + EPS) * g + b

    x = np.asarray(tgt, np.float64)
    b_, n, d = x.shape
    dk = d // H
    q = (x @ Wq.T).reshape(b_, n, H, dk).transpose(0, 2, 1, 3)
    k = (x @ Wk.T).reshape(b_, n, H, dk).transpose(0, 2, 1, 3)
    v = (x @ Wv.T).reshape(b_, n, H, dk).transpose(0, 2, 1, 3)
    s = np.einsum("bhqd,bhkd->bhqk", q, k) / np.sqrt(dk)
    mask = np.asarray(tgt_mask)[None, None] | \
        np.asarray(tgt_key_pad_mask)[:, None, None, :]
    s = np.where(mask, -np.inf, s)
    s = s - s.max(-1, keepdims=True)
    e = np.exp(s)
    att = e / e.sum(-1, keepdims=True)
    o = np.einsum("bhqk,bhkd->bhqd", att, v).transpose(0, 2, 1, 3).reshape(
        b_, n, d)
    h = o @ Wo.T + bo
    x1 = ln(x + h, g1, beta1)
    gl = x1 @ W1.T + b1
    gl = 0.5 * gl * (1 + np.tanh(np.sqrt(2 / np.pi) * (gl + 0.044715 * gl**3)))
    m = gl @ W2.T + b2
    return ln(x1 + m, g2, beta2).astype(np.float32)


def kernel(**inputs):
    in_maps = make_in_maps(**inputs)
    if in_maps is None:
        return _numpy_reference(**inputs)
    run = _get_runner()
    results = run(in_maps)
    out = np.empty((B, N, D), np.float32)
    for g in range(NCORES):
        b, r = g // TPR, g % TPR
        out[b, TS * r:TS * (r + 1), :] = results[g]["out"].T
    return out


# revision 31
# speedup vs baseline: 421.7501x; 421.7501x over previous
"""Trainium2 Bass kernel for a causal self-attention transformer layer.

Layer (PostNorm, eval):
    h  = MHA_causal(tgt); x = LN(tgt + h); out = LN(x + gelu(x@W1.T+b1)@W2.T+b2)
Shapes: B=2, N=2048, D=1024, H=16 (dk=64), FFN=4096.

Distribution over 8 NeuronCores (core g: batch b=g//4, rank r=g%4):
  Stage A (head-parallel): core computes Q/K/V + causal attention for its 4
    heads over all 2048 tokens of its batch. Activations are kept
    feature-major [feat, tok] so every matmul contracts on partitions.
    Softmax uses no max-subtraction (scores are O(1) here); the denominator
    is produced for free as an extra all-ones column of V in the AV matmul.
  AllToAll (8 cores) converts head-sharded attention output into
    token-sharded (512 tokens/core). Cross-batch slots carry duplicate data
    which is nulled by zero-padded Wo weights.
  Stage B (token-parallel): Wo + bias + residual, LN1, W1+gelu, W2 + bias +
    residual, LN2 for the core's 512 tokens. LayerNorm over the feature
    (partition) axis uses ones-vector matmuls for the sums and K=1
    broadcast matmuls (float32r) to spread the per-token mean/rstd.

Host side shards/transposes/casts inputs, runs the SPMD program via a
cached PJRT callable, and reassembles the full [2, 2048, 1024] output.
"""

import numpy as np
import ml_dtypes

import concourse.bass as bass
import concourse.mybir as mybir
import concourse.tile as tile
from concourse.vector_clock import ScopedClock

BF16 = mybir.dt.bfloat16
F32 = mybir.dt.float32
F32R = mybir.dt.float32r
AF = mybir.ActivationFunctionType
ALU = mybir.AluOpType

B, N, D, H, DK, FFN = 2, 2048, 1024, 16, 64, 4096
EPS = 1e-5
NCORES = 8
TPR = 4            # tensor-parallel ranks per batch
HL = H // TPR      # heads per core (4)
DVL = HL * DK      # local head width (256)
TS = N // TPR      # tokens per core in stage B (512)
NP = 128           # partitions
QT = 512           # q tile width
NKB = N // NP      # key blocks (16)

# ---------------------------------------------------------------------------
# Walrus in this environment encodes at most ONE sync-wait per instruction.
# Patch Tile's exit drain and post-split every multi-wait instruction.
# ---------------------------------------------------------------------------

_wsplit = [0]


def _patched_drain_and_barrier(self, tick_clock, wait_clock):
    nc = self.nc
    probe = nc.sync.nop(nofuse=True)
    wait_clock.add_sem_waits(probe.ins, ScopedClock({None: tick_clock.global_clock}))
    si = probe.ins.sync_info
    waits = list(si.on_wait) if si is not None else []
    if waits:
        probe.ins.sync_info = mybir.SyncInfo(on_wait=[waits[0]], on_update=[])
        for w in waits[1:]:
            extra = nc.sync.nop(nofuse=True)
            extra.ins.sync_info = mybir.SyncInfo(on_wait=[w], on_update=[])
    nc.sync.drain()
    nc.all_engine_barrier()
    popped = nc._tile_sem_poison_stack.pop()
    assert popped is self._sem_poison
    nc.clear_and_free_semaphores(list(self.sems.allocated().values()))
    nc.all_engine_barrier()


tile.TileContext._drain_and_barrier = _patched_drain_and_barrier


def _split_multiwait_instructions(nc):
    for fn in nc.m.functions:
        for bb in fn.blocks:
            insts = bb.instructions
            if not any(
                i.sync_info is not None and len(i.sync_info.on_wait) > 1
                for i in insts
            ):
                continue
            new = []
            for inst in insts:
                si = inst.sync_info
                waits = list(si.on_wait) if si is not None else []
                if len(waits) > 1:
                    for w in waits[:-1]:
                        _wsplit[0] += 1
                        new.append(mybir.InstNoOp(
                            name=f"wsplit-{_wsplit[0]}",
                            engine=inst.engine,
                            sync_info=mybir.SyncInfo(on_wait=[w], on_update=[]),
                        ))
                    inst.sync_info = mybir.SyncInfo(
                        on_wait=[waits[-1]], on_update=list(si.on_update)
                    )
                new.append(inst)
            bb.instructions = new


# ---------------------------------------------------------------------------
# Program builder
# ---------------------------------------------------------------------------

def _build_program(for_sim=False, split=True):
    nc = bass.Bass("TRN2", target_bir_lowering=False, debug=False,
                   num_devices=NCORES)

    def din(name, shape, dt):
        return nc.dram_tensor(name, shape, dt, kind="ExternalInput").ap()

    xt_d = din("xt", [D, N], BF16)            # tgt[b].T
    xtr_d = din("xtr", [D, TS], F32)          # residual slice (my tokens)
    wqt_d = din("wqt", [D, DVL], BF16)        # Wq[local].T
    wkt_d = din("wkt", [D, DVL], BF16)
    wvt_d = din("wvt", [D, DVL], BF16)
    wot_d = din("wot", [2 * D, D], BF16)      # zero-padded Wo.T (A2A slots)
    w1t_d = din("w1t", [D, FFN], BF16)
    w2t_d = din("w2t", [FFN, D], BF16)
    bo_d = din("bo", [NP, D // NP], F32)
    b1_d = din("b1", [NP, FFN // NP], F32)
    b2_d = din("b2", [NP, D // NP], F32)
    mask_d = din("mask2", [NP, 4, 2 * QT], BF16)
    out_d = nc.dram_tensor("out", [D, TS], F32, kind="ExternalOutput").ap()

    HTS = TS // 2
    cc_in = [nc.dram_tensor(f"cc_in{i}", [NCORES * DVL, HTS], BF16).ap()
             for i in range(2)]
    cc_out = [nc.dram_tensor(f"cc_out{i}", [NCORES * DVL, HTS], BF16).ap()
              for i in range(2)]
    rb_d = nc.dram_tensor("rb_bounce", [4 * NCORES // 2 * 2, QT], F32).ap()

    with tile.TileContext(nc, num_cores=NCORES) as tc:
        with tc.tile_pool(name="const", bufs=1) as cpool:
            onesf = cpool.tile([NP, NP], F32)
            nc.vector.memset(onesf[:], 1.0)
            ones64r = cpool.tile([NP, 64], F32R)   # row 64 used by AV bcast
            nc.vector.tensor_copy(ones64r[64:65, :], onesf[64:65, 0:64])
            ones128r = cpool.tile([1, NP], F32R)   # LN bcast lhsT
            nc.vector.tensor_copy(ones128r[:], onesf[0:1, :])
            ones128b = cpool.tile([NP, 1], BF16)   # LN stats lhsT
            nc.vector.memset(ones128b[:], 1.0)
            onescol_r = cpool.tile([NP, 1], F32R)  # f32r variant
            nc.vector.tensor_copy(onescol_r[:], onesf[:, 0:1])
            bo_sb = cpool.tile([NP, D // NP], F32)
            nc.sync.dma_start(out=bo_sb[:], in_=bo_d[:])
            b1_sb = cpool.tile([NP, FFN // NP], F32)
            nc.sync.dma_start(out=b1_sb[:], in_=b1_d[:])
            b2_sb = cpool.tile([NP, D // NP], F32)
            nc.sync.dma_start(out=b2_sb[:], in_=b2_d[:])
            mask_sb = cpool.tile([NP, 4, 2 * QT], BF16)
            nc.sync.dma_start(out=mask_sb[:], in_=mask_d[:])

            # stage-B weight tiles allocated up front; their DMAs are
            # emitted after the attention-critical loads so they fill
            # spare DMA bandwidth during attention
            prefetch = tc.tile_pool(name="prefetch", bufs=1)
            pf = prefetch.__enter__()
            wot_sb = pf.tile([NP, 2 * D // NP, D], BF16)
            xtr_sb = pf.tile([NP, D // NP, TS], F32)

            # ---------------- Stage A: QKV + attention (4 local heads) ----
            with tc.tile_pool(name="sa", bufs=1) as sa:
                wq_sb = sa.tile([NP, D // NP, DVL], BF16)
                nc.sync.dma_start(
                    out=wq_sb[:], in_=wqt_d.rearrange("(c p) f -> p c f", p=NP))
                wk_sb = sa.tile([NP, D // NP, DVL], BF16)
                nc.sync.dma_start(
                    out=wk_sb[:], in_=wkt_d.rearrange("(c p) f -> p c f", p=NP))
                wv_sb = sa.tile([NP, D // NP, DVL], BF16)
                nc.sync.dma_start(
                    out=wv_sb[:], in_=wvt_d.rearrange("(c p) f -> p c f", p=NP))
                xt_c = []
                for dc in range(D // NP):
                    xc = sa.tile([NP, N], BF16, tag=f"xt{dc}", name=f"xt{dc}")
                    nc.sync.dma_start(
                        out=xc[:], in_=xt_d[dc * NP:(dc + 1) * NP, :])
                    xt_c.append(xc)

                q_sb = sa.tile([NP, 2, N], BF16)
                k_sb = sa.tile([NP, 2, N], BF16)
                v_sb = sa.tile([NP, NKB, HL * (DK + 1)], BF16)
                nc.sync.dma_start(
                    out=wot_sb[:],
                    in_=wot_d.rearrange("(c p) f -> p c f", p=NP))
                nc.sync.dma_start(
                    out=xtr_sb[:],
                    in_=xtr_d.rearrange("(c p) t -> p c t", p=NP))

                with tc.tile_pool(name="qkv_ps", bufs=1, space="PSUM") as qp:
                    for o in range(2):
                        for t in range(N // QT):
                            ps = qp.tile([NP, QT], F32, tag="qk", bufs=3)
                            for dc in range(D // NP):
                                nc.tensor.matmul(
                                    ps[:],
                                    wq_sb[:, dc, o * NP:(o + 1) * NP],
                                    xt_c[dc][:, t * QT:(t + 1) * QT],
                                    start=(dc == 0), stop=(dc == D // NP - 1))
                            nc.scalar.activation(
                                q_sb[:, o, t * QT:(t + 1) * QT], ps[:], AF.Copy)
                            ps2 = qp.tile([NP, QT], F32, tag="qk", bufs=3)
                            for dc in range(D // NP):
                                nc.tensor.matmul(
                                    ps2[:],
                                    wk_sb[:, dc, o * NP:(o + 1) * NP],
                                    xt_c[dc][:, t * QT:(t + 1) * QT],
                                    start=(dc == 0), stop=(dc == D // NP - 1))
                            nc.vector.tensor_copy(
                                k_sb[:, o, t * QT:(t + 1) * QT], ps2[:])
                    for t in range(NKB):
                        psv = qp.tile([NP, DVL], F32, tag="v", bufs=2)
                        for dc in range(D // NP):
                            nc.tensor.matmul(
                                psv[:],
                                xt_c[dc][:, t * NP:(t + 1) * NP],
                                wv_sb[:, dc, :],
                                start=(dc == 0), stop=(dc == D // NP - 1))
                        vview = v_sb[:, t, :].rearrange("p (h c) -> p h c", c=DK + 1)
                        nc.vector.tensor_copy(
                            vview[:, :, 0:DK],
                            psv[:].rearrange("p (h c) -> p h c", c=DK))
                        nc.vector.memset(vview[:, :, DK:DK + 1], 1.0)

                # attention: per q-tile j, per head-pair hp
                attn_all = sa.tile([DK, HL * N], BF16)
                attn_h = [attn_all[:, h * N:(h + 1) * N] for h in range(HL)]
                with tc.tile_pool(name="att_ps", bufs=1, space="PSUM") as ap:
                    for j in range(N // QT):
                        for hp in range(2):
                            h0, h1 = 2 * hp, 2 * hp + 1
                            pav0 = ap.tile([NP, QT], F32, tag="av0", bufs=2)
                            pav1 = ap.tile([NP, QT], F32, tag="av1", bufs=2)
                            nkb = (j + 1) * (QT // NP)
                            for kb in range(nkb):
                                ps_s = ap.tile([NP, 2 * QT], F32, tag="s", bufs=2)
                                nc.tensor.matmul(
                                    ps_s[:, 0:QT],
                                    k_sb[0:64, hp, kb * NP:(kb + 1) * NP],
                                    q_sb[0:64, hp, j * QT:(j + 1) * QT],
                                    start=True, stop=True)
                                nc.tensor.matmul(
                                    ps_s[:, QT:2 * QT],
                                    k_sb[64:NP, hp, kb * NP:(kb + 1) * NP],
                                    q_sb[64:NP, hp, j * QT:(j + 1) * QT],
                                    start=True, stop=True)
                                e_sb = sa.tile([NP, 2 * QT], BF16, tag="e",
                                               bufs=3)
                                nc.scalar.activation(
                                    e_sb[:], ps_s[:], AF.Exp,
                                    scale=1.0 / np.sqrt(DK))
                                di = kb - (QT // NP) * j
                                if di >= 0:
                                    nc.vector.tensor_tensor(
                                        e_sb[:], e_sb[:], mask_sb[:, di, :],
                                        op=ALU.mult)
                                nc.tensor.matmul(
                                    pav0[0:DK + 1, :],
                                    v_sb[:, kb, h0 * (DK + 1):(h0 + 1) * (DK + 1)],
                                    e_sb[:, 0:QT],
                                    start=(kb == 0), stop=(kb == nkb - 1))
                                nc.tensor.matmul(
                                    pav1[0:DK + 1, :],
                                    v_sb[:, kb, h1 * (DK + 1):(h1 + 1) * (DK + 1)],
                                    e_sb[:, QT:2 * QT],
                                    start=(kb == 0), stop=(kb == nkb - 1))
                            for e01, pav, h in ((0, pav0, h0), (1, pav1, h1)):
                                site = (j * 2 + hp) * 2 + e01
                                rr = sa.tile([NP, QT], F32, tag="rr", bufs=2)
                                nc.vector.reciprocal(
                                    rr[64:65, :], pav[DK:DK + 1, :])
                                nc.sync.dma_start(
                                    out=rb_d[site:site + 1, :],
                                    in_=rr[64:65, :])
                                rbc = sa.tile([DK, QT], F32, tag="rbc", bufs=3)
                                nc.sync.dma_start(
                                    out=rbc[:],
                                    in_=rb_d[site:site + 1, :]
                                    .partition_broadcast(DK))
                                nc.vector.tensor_tensor(
                                    attn_h[h][:, j * QT:(j + 1) * QT],
                                    pav[0:DK, :], rbc[:], op=ALU.mult)
                        # stage chunk j of all 4 heads into cc_in slots
                        # j and j+4 of both halves
                        for h in range(HL):
                            for i in range(2):
                                src = attn_all[:, h * N + j * QT + i * HTS:
                                               h * N + j * QT + (i + 1) * HTS]
                                for s in (j, j + TPR):
                                    nc.sync.dma_start(
                                        out=cc_in[i][DVL * s + DK * h:
                                                     DVL * s + DK * (h + 1), :],
                                        in_=src)

            for i in range(2):
                if for_sim:
                    # stand-in for the A2A so the single-core timeline sim
                    # runs: same bytes through the DMA path
                    nc.sync.dma_start(out=cc_out[i][:], in_=cc_in[i][:])
                else:
                    nc.gpsimd.collective_compute(
                        "AllToAll", ALU.bypass,
                        ins=[cc_in[i][:]], outs=[cc_out[i][:]],
                        replica_groups=[list(range(NCORES))],
                    )

            # ---------------- Stage B: Wo + LN1 + MLP + LN2, two ------
            # 256-token halves pipelined against the two AllToAlls
            def _sel(x, ob):
                return x[ob][:] if isinstance(x, list) else x[:, ob, :]

            def layer_norm(tc, pool, src, outf, outb, W):
                """src [NP, 8, W] f32; outf/outb tensors or per-ob lists."""
                nblk = D // NP
                sbf = pool.tile([NP, nblk, W], BF16, tag="ln_bf")
                sqb = pool.tile([NP, nblk, W], BF16, tag="ln_sq")
                with tc.tile_pool(name="ln_ps", bufs=1, space="PSUM") as lp:
                    pmu = lp.tile([1, W], F32, tag="mu")
                    psq = lp.tile([1, W], F32, tag="sq")
                    for ob in range(nblk):
                        nc.vector.tensor_copy(sbf[:, ob, :], src[:, ob, :])
                        nc.vector.tensor_tensor(
                            sqb[:, ob, :], sbf[:, ob, :], sbf[:, ob, :],
                            op=ALU.mult)
                        nc.tensor.matmul(pmu[:], ones128b[:], sbf[:, ob, :],
                                         start=(ob == 0), stop=(ob == nblk - 1))
                        nc.tensor.matmul(psq[:], ones128b[:], sqb[:, ob, :],
                                         start=(ob == 0), stop=(ob == nblk - 1))
                    rows = pool.tile([1, 7, W], F32, tag="ln_rows")
                    mu, ex2, mu2, var = (rows[:, i, :] for i in range(4))
                    vr, rstd, brow = (rows[:, i, :] for i in range(4, 7))
                    nc.vector.tensor_scalar_mul(mu, pmu, 1.0 / D)
                    nc.vector.tensor_scalar_mul(ex2, psq[:], 1.0 / D)
                    nc.vector.tensor_tensor(mu2, mu, mu, op=ALU.mult)
                    nc.vector.tensor_tensor(var, ex2, mu2, op=ALU.subtract)
                    nc.vector.tensor_scalar_add(var, var, EPS)
                    nc.vector.reciprocal(vr, var)
                    nc.scalar.activation(rstd, vr, AF.Sqrt)
                    nc.vector.tensor_tensor(brow, mu, rstd, op=ALU.mult)
                    rowr = pool.tile([1, 2, W], F32R, tag="ln_rowr")
                    nc.vector.tensor_copy(rowr[:, 0, :], rstd)
                    nc.vector.tensor_scalar_mul(rowr[:, 1, :], brow, -1.0)
                    pA = lp.tile([NP, W], F32, tag="bA")
                    pB = lp.tile([NP, W], F32, tag="bB")
                    pA, pB = pA[:], pB[:]
                    nc.tensor.matmul(pA, ones128r[:], rowr[:, 0, :],
                                     start=True, stop=True)
                    nc.tensor.matmul(pB, ones128r[:], rowr[:, 1, :],
                                     start=True, stop=True)
                    A_sb = pool.tile([NP, W], F32, tag="ln_A")
                    B_sb = pool.tile([NP, W], F32, tag="ln_B")
                    nc.scalar.activation(A_sb[:], pA, AF.Copy)
                    nc.scalar.activation(B_sb[:], pB, AF.Copy)
                    if outb is not None:
                        # bf16 fast path first so downstream matmuls unblock
                        Ab = pool.tile([NP, W], BF16, tag="ln_Ab")
                        Bb = pool.tile([NP, W], BF16, tag="ln_Bb")
                        nc.scalar.activation(Ab[:], pA, AF.Copy)
                        nc.scalar.activation(Bb[:], pB, AF.Copy)
                        tmpb = pool.tile([NP, nblk, W], BF16, tag="ln_tmpb")
                        for ob in range(nblk):
                            nc.vector.tensor_tensor(
                                tmpb[:, ob, :], _sel(src, ob), Ab[:],
                                op=ALU.mult)
                            nc.vector.tensor_tensor(
                                _sel(outb, ob), tmpb[:, ob, :], Bb[:],
                                op=ALU.add)
                    tmp = pool.tile([NP, nblk, W], F32, tag="ln_tmp")
                    for ob in range(nblk):
                        nc.vector.tensor_tensor(
                            tmp[:, ob, :], src[:, ob, :], A_sb[:], op=ALU.mult)
                        nc.vector.tensor_tensor(
                            _sel(outf, ob), tmp[:, ob, :], B_sb[:], op=ALU.add)

            with tc.tile_pool(name="sbw", bufs=1) as sbw:
                x2f = [sbw.tile([NP, TS], F32, tag=f"x2f{ob}",
                                name=f"x2f{ob}") for ob in range(D // NP)]
                x2b = [sbw.tile([NP, TS], BF16, tag=f"x2b{ob}",
                                name=f"x2b{ob}") for ob in range(D // NP)]

                def wo_half(h01, wp, sbo):
                    hsl = slice(h01 * HTS, (h01 + 1) * HTS)
                    ao = sbo.tile([NP, 2 * D // NP, HTS], BF16, tag="ao",
                                  name=f"ao_{h01}")
                    nc.sync.dma_start(
                        out=ao[:],
                        in_=cc_out[h01].rearrange("(c p) t -> p c t", p=NP))
                    sum1 = sbo.tile([NP, D // NP, HTS], F32R, tag="sum1",
                                    name=f"sum1_{h01}")
                    for ob in range(D // NP):
                        ph = wp.tile([NP, HTS], F32, tag=f"m{h01}", bufs=2)
                        for c in range(2 * D // NP):
                            nc.tensor.matmul(
                                ph[:], wot_sb[:, c, ob * NP:(ob + 1) * NP],
                                ao[:, c, :],
                                start=(c == 0), stop=(c == 2 * D // NP - 1))
                        hb = sbo.tile([NP, HTS], F32, tag=f"hb{h01}", bufs=2)
                        nc.scalar.activation(hb[:], ph[:], AF.Identity,
                                             bias=bo_sb[:, ob:ob + 1])
                        nc.vector.tensor_tensor(
                            sum1[:, ob, :], hb[:], xtr_sb[:, ob, hsl],
                            op=ALU.add)
                    return sum1

                def ln_stats(pool, lp, src, W):
                    nblk = D // NP
                    sqb = pool.tile([NP, nblk, W], F32R, tag="ln_sq")
                    pmu = lp.tile([1, W], F32, tag="statmu")
                    psq = lp.tile([1, W], F32, tag="statsq")
                    pmu, psq = pmu[:], psq[:]
                    for ob in range(nblk):
                        nc.vector.tensor_tensor(
                            sqb[:, ob, :], src[:, ob, :], src[:, ob, :],
                            op=ALU.mult)
                        nc.tensor.matmul(pmu, onescol_r[:], src[:, ob, :],
                                         start=(ob == 0), stop=(ob == nblk - 1))
                        nc.tensor.matmul(psq, onescol_r[:], sqb[:, ob, :],
                                         start=(ob == 0), stop=(ob == nblk - 1))
                    rows = pool.tile([1, 4, W], F32, tag="ln_rows")
                    mu, ex2e, mu2, vr = (rows[:, i, :] for i in range(4))
                    nc.vector.tensor_scalar_mul(mu, pmu, 1.0 / D)
                    # ex2e = psq/D + EPS in one fused op
                    nc.vector.tensor_scalar(ex2e, psq, 1.0 / D, EPS,
                                            op0=ALU.mult, op1=ALU.add)
                    nc.vector.tensor_tensor(mu2, mu, mu, op=ALU.mult)
                    nc.vector.tensor_tensor(mu2, ex2e, mu2, op=ALU.subtract)
                    nc.vector.reciprocal(vr, mu2)
                    rowr = pool.tile([1, 2, W], F32R, tag="ln_rowr")
                    nc.scalar.activation(rowr[:, 0, :], vr, AF.Sqrt)
                    # rowr1 = (mu * -1) * rstd in one fused op
                    nc.vector.scalar_tensor_tensor(
                        rowr[:, 1, :], mu, -1.0, rowr[:, 0, :],
                        op0=ALU.mult, op1=ALU.mult)
                    return None, rowr

                def ln_finish(pool, lp, src, sbf, rowr, outf, outb, W):
                    nblk = D // NP
                    pA = lp.tile([NP, W], F32, tag="bA")
                    pB = lp.tile([NP, W], F32, tag="bB")
                    pA, pB = pA[:], pB[:]
                    nc.tensor.matmul(pA, ones128r[:], rowr[:, 0, :],
                                     start=True, stop=True)
                    nc.tensor.matmul(pB, ones128r[:], rowr[:, 1, :],
                                     start=True, stop=True)
                    A_sb = pool.tile([NP, W], F32, tag="ln_A")
                    B_sb = pool.tile([NP, W], F32, tag="ln_B")
                    nc.scalar.activation(A_sb[:], pA, AF.Copy)
                    nc.scalar.activation(B_sb[:], pB, AF.Copy)
                    if outb is not None:
                        Ab = pool.tile([NP, W], BF16, tag="ln_Ab")
                        Bb = pool.tile([NP, W], BF16, tag="ln_Bb")
                        nc.scalar.activation(Ab[:], pA, AF.Copy)
                        nc.scalar.activation(Bb[:], pB, AF.Copy)
                        tmpb = pool.tile([NP, nblk, W], BF16, tag="ln_tmpb")
                        for ob in range(nblk):
                            eng = nc.gpsimd if ob % 3 == 2 else nc.vector
                            eng.tensor_tensor(
                                tmpb[:, ob, :], _sel(src, ob), Ab[:],
                                op=ALU.mult)
                            nc.vector.tensor_tensor(
                                outb[ob], tmpb[:, ob, :], Bb[:], op=ALU.add)

                    def fp32_path():
                        tmp = pool.tile([NP, nblk, W], F32, tag="ln_tmp")
                        for ob in range(nblk):
                            eng = nc.gpsimd if (outb is None and ob >= 5) \
                                else nc.vector
                            eng.tensor_tensor(
                                tmp[:, ob, :], _sel(src, ob), A_sb[:],
                                op=ALU.mult)
                            nc.vector.tensor_tensor(
                                outf[ob], tmp[:, ob, :], B_sb[:], op=ALU.add)
                    return fp32_path

                def _sel(x, ob):
                    return x[ob][:] if isinstance(x, list) else x[:, ob, :]

                hs = [slice(0, HTS), slice(HTS, TS)]
                g_all = sbw.tile([NP, FFN // NP, TS], BF16)
                with tc.tile_pool(name="sbo", bufs=1) as sbo, \
                     tc.tile_pool(name="ln1p", bufs=1) as lnp, \
                     tc.tile_pool(name="ln1_ps", bufs=1, space="PSUM") as lp, \
                     tc.tile_pool(name="w1s", bufs=1) as w1s, \
                     tc.tile_pool(name="w1_ps", bufs=1, space="PSUM") as mp:
                    wp = mp
                    # interleaved emission: PE stays busy on the other half
                    # while DVE/ACT run each half's LN row chain
                    sum1_0 = wo_half(0, wp, sbo)
                    sum1_1 = wo_half(1, wp, sbo)
                    sbf0, rowr0 = ln_stats(lnp, lp, sum1_0, HTS)
                    fp0 = ln_finish(lnp, lp, sum1_0, sbf0, rowr0,
                                    [x[:, hs[0]] for x in x2f],
                                    [x[:, hs[0]] for x in x2b], HTS)
                    sbf1, rowr1 = ln_stats(lnp, lp, sum1_1, HTS)
                    fp1 = ln_finish(lnp, lp, sum1_1, sbf1, rowr1,
                                    [x[:, hs[1]] for x in x2f],
                                    [x[:, hs[1]] for x in x2b], HTS)

                    # W1 + gelu: single weight stream; half-1 lags 2 chunks
                    w1cs = {}
                    LAG = 2
                    def w1_group(fc, h01):
                        w1c = w1cs[fc]
                        for fs in range(QT // NP):
                            fb = fc * (QT // NP) + fs
                            pm = mp.tile([NP, HTS], F32, tag=f"m{h01}",
                                         bufs=2)
                            for dc in range(D // NP):
                                nc.tensor.matmul(
                                    pm[:], w1c[:, dc, fs * NP:(fs + 1) * NP],
                                    x2b[dc][:, hs[h01]],
                                    start=(dc == 0), stop=(dc == D // NP - 1))
                            nc.scalar.activation(
                                g_all[:, fb, hs[h01]], pm[:],
                                AF.Sigmoid if for_sim else AF.Gelu_apprx_tanh,
                                bias=b1_sb[:, fb:fb + 1])
                    for fc in range(FFN // QT + LAG):
                        if fc == 1:
                            # x2f fp32 chains run on DVE while W1 streams
                            fp0()
                            fp1()
                        if fc < FFN // QT:
                            w1c = w1s.tile([NP, D // NP, QT], BF16, tag="w1c",
                                           bufs=LAG + 1, name=f"w1c{fc}")
                            nc.sync.dma_start(
                                out=w1c[:],
                                in_=w1t_d[:, fc * QT:(fc + 1) * QT].rearrange(
                                    "(c p) f -> p c f", p=NP))
                            w1cs[fc] = w1c
                            w1_group(fc, 0)
                        if fc >= LAG:
                            w1_group(fc - LAG, 1)
                            del w1cs[fc - LAG]

                # W2 full-width: one weight stream, 8 psum accumulators
                with tc.tile_pool(name="w2s", bufs=1) as w2s:
                    sum2 = w2s.tile([NP, D // NP, TS], F32R)
                    yp_cm = tc.tile_pool(name="w2_ps", bufs=1, space="PSUM")
                    yp = yp_cm.__enter__()
                    pys = [yp.tile([NP, TS], F32, tag=f"y{ob}", name=f"y{ob}")
                           for ob in range(D // NP)]
                    for fb in range(FFN // NP):
                        w2c = w2s.tile([NP, D], BF16, tag="w2c", bufs=3)
                        nc.sync.dma_start(
                            out=w2c[:], in_=w2t_d[fb * NP:(fb + 1) * NP, :])
                        for ob in range(D // NP):
                            nc.tensor.matmul(
                                pys[ob][:], w2c[:, ob * NP:(ob + 1) * NP],
                                g_all[:, fb, :],
                                start=(fb == 0), stop=(fb == FFN // NP - 1))
                    for ob in range(D // NP):
                        mb = w2s.tile([NP, TS], F32, tag="mb", bufs=2)
                        nc.scalar.activation(mb[:], pys[ob][:], AF.Identity,
                                             bias=b2_sb[:, ob:ob + 1])
                        nc.vector.tensor_tensor(
                            sum2[:, ob, :], mb[:], x2f[ob][:], op=ALU.add)
                    yp_cm.__exit__(None, None, None)

                    # LN2 full width
                    yf = w2s.tile([NP, D // NP, TS], F32)
                    with tc.tile_pool(name="ln2", bufs=1) as lnp2, \
                         tc.tile_pool(name="ln2_ps", bufs=1,
                                      space="PSUM") as lp2:
                        fp2 = ln_finish(lnp2, lp2, sum2,
                                        *ln_stats(lnp2, lp2, sum2, TS),
                                        [yf[:, ob, :]
                                         for ob in range(D // NP)],
                                        None, TS)
                        fp2()
                    for ob in range(D // NP):
                        nc.sync.dma_start(
                            out=out_d[ob * NP:(ob + 1) * NP, :],
                            in_=yf[:, ob, :])
            prefetch.__exit__(None, None, None)

    if split:
        _split_multiwait_instructions(nc)
    return nc


# ---------------------------------------------------------------------------
# Cached PJRT runner (mirrors bass2jax.run_bass_via_pjrt multi-core path but
# keeps the jitted callable so repeat calls don't recompile).
# ---------------------------------------------------------------------------

_RUNNER = None


def _make_runner(nc):
    import jax
    from jax.sharding import Mesh, PartitionSpec
    from jax.experimental.shard_map import shard_map
    from concourse import bass2jax

    bass2jax.install_neuronx_cc_hook()
    partition_name = (nc.partition_id_tensor.name
                      if nc.partition_id_tensor else None)
    in_names, out_names, out_avals = [], [], []
    for alloc in nc.m.functions[0].allocations:
        if not isinstance(alloc, mybir.MemoryLocationSet):
            continue
        name = alloc.memorylocations[0].name
        if alloc.kind == "ExternalInput":
            if name != partition_name:
                in_names.append(name)
        elif alloc.kind == "ExternalOutput":
            out_names.append(name)
            out_avals.append(jax.core.ShapedArray(
                tuple(alloc.tensor_shape), mybir.dt.np(alloc.dtype)))
    n_params = len(in_names)
    all_in_names = list(in_names) + list(out_names)
    if partition_name is not None:
        all_in_names.append(partition_name)
    donate = tuple(range(n_params, n_params + len(out_names)))

    def _body(*args):
        operands = list(args)
        if partition_name is not None:
            operands.append(bass2jax.partition_id_tensor())
        outs = bass2jax._bass_exec_p.bind(
            *operands,
            out_avals=tuple(out_avals),
            in_names=tuple(all_in_names),
            out_names=tuple(out_names),
            lowering_input_output_aliases=(),
            sim_require_finite=True,
            sim_require_nnan=True,
            nc=nc,
        )
        return tuple(outs)

    devices = jax.devices()[:NCORES]
    mesh = Mesh(np.asarray(devices), ("core",))
    specs = (PartitionSpec("core"),) * (n_params + len(out_names))
    sharded = jax.jit(
        shard_map(_body, mesh=mesh, in_specs=specs,
                  out_specs=(PartitionSpec("core"),) * len(out_names),
                  check_rep=False),
        donate_argnums=donate, keep_unused=True)

    from jax.sharding import NamedSharding
    shard = NamedSharding(mesh, PartitionSpec("core"))

    def prepare(in_maps):
        per_core = [[np.asarray(m[name]) for name in in_names]
                    for m in in_maps]
        concat_in = [np.concatenate([per_core[c][i] for c in range(NCORES)],
                                    axis=0) for i in range(n_params)]
        return [jax.device_put(a, shard) for a in concat_in]

    def run_prepared(dev_in, materialize=True):
        concat_zeros = [np.zeros((NCORES * a.shape[0], *a.shape[1:]), a.dtype)
                        for a in out_avals]
        out_arrs = sharded(*dev_in, *concat_zeros)
        if not materialize:
            jax.block_until_ready(out_arrs)
            return None
        return [
            {name: np.asarray(out_arrs[i]).reshape(
                NCORES, *out_avals[i].shape)[c]
             for i, name in enumerate(out_names)}
            for c in range(NCORES)
        ]

    def run_prepared_async(dev_in):
        concat_zeros = [np.zeros((NCORES * a.shape[0], *a.shape[1:]), a.dtype)
                        for a in out_avals]
        return sharded(*dev_in, *concat_zeros)

    def run(in_maps, materialize=True):
        return run_prepared(prepare(in_maps), materialize)

    run.prepare = prepare
    run.run_prepared = run_prepared
    run.run_prepared_async = run_prepared_async
    return run


def _get_runner():
    global _RUNNER
    if _RUNNER is None:
        nc = _build_program()
        _RUNNER = _make_runner(nc)
    return _RUNNER


# ---------------------------------------------------------------------------
# Host-side sharding / gathering
# ---------------------------------------------------------------------------

def _bf16(a):
    return np.ascontiguousarray(a).astype(ml_dtypes.bfloat16)


def make_in_maps(tgt, tgt_mask, tgt_key_pad_mask, Wq, Wk, Wv, Wo, bo,
                 W1, b1, W2, b2, g1, beta1, g2, beta2):
    causal = np.triu(np.ones((N, N), bool), k=1)
    if not (np.array_equal(np.asarray(tgt_mask), causal)
            and not np.asarray(tgt_key_pad_mask).any()
            and np.allclose(np.asarray(g1), 1) and np.allclose(np.asarray(g2), 1)
            and np.allclose(np.asarray(beta1), 0)
            and np.allclose(np.asarray(beta2), 0)):
        return None  # unsupported masking/affine -> numpy fallback

    tgt = np.asarray(tgt, np.float32)
    Wq, Wk, Wv, Wo = (np.asarray(a, np.float32) for a in (Wq, Wk, Wv, Wo))
    W1, W2 = np.asarray(W1, np.float32), np.asarray(W2, np.float32)
    bo, b1, b2 = (np.asarray(a, np.float32) for a in (bo, b1, b2))

    # diagonal-block multiplicative masks [128, 4, 1024]
    q_idx = np.arange(QT)[None, :]
    mask2 = np.empty((NP, 4, 2 * QT), np.float32)
    for i in range(4):
        m = (NP * i + np.arange(NP)[:, None] <= q_idx).astype(np.float32)
        mask2[:, i, :] = np.concatenate([m, m], axis=1)

    w1t = _bf16(W1.T)
    w2t = _bf16(W2.T)
    bo_c = np.ascontiguousarray(bo.reshape(D // NP, NP).T)
    b1_c = np.ascontiguousarray(b1.reshape(FFN // NP, NP).T)
    b2_c = np.ascontiguousarray(b2.reshape(D // NP, NP).T)
    mask2_bf = _bf16(mask2)

    xt_b = [_bf16(tgt[b].T) for b in range(B)]
    in_maps = []
    for g in range(NCORES):
        b, r = g // TPR, g % TPR
        sl = slice(DVL * r, DVL * (r + 1))
        wot_ext = np.zeros((2 * D, D), np.float32)
        wot_ext[D * b:D * (b + 1), :] = Wo.T
        in_maps.append({
            "xt": xt_b[b],
            "xtr": np.ascontiguousarray(tgt[b].T[:, TS * r:TS * (r + 1)]),
            "wqt": _bf16(Wq[sl, :].T),
            "wkt": _bf16(Wk[sl, :].T),
            "wvt": _bf16(Wv[sl, :].T),
            "wot": _bf16(wot_ext),
            "w1t": w1t,
            "w2t": w2t,
            "bo": bo_c,
            "b1": b1_c,
            "b2": b2_c,
            "mask2": mask2_bf,
        })
    return in_maps


def _numpy_reference(tgt, tgt_mask, tgt_key_pad_mask, Wq, Wk, Wv, Wo, bo,
                     W1, b1, W2, b2, g1, beta1, g2, beta2):
    def ln(x, g, b):
        mu = x.mean(-1, keepdims=True)
        var = ((x - mu) ** 2).mean(-1, keepdims=True)
        return (x - mu) / np.sqrt(var + EPS) * g + b

    x = np.asarray(tgt, np.float64)
    b_, n, d = x.shape
    dk = d // H
    q = (x @ Wq.T).reshape(b_, n, H, dk).transpose(0, 2, 1, 3)
    k = (x @ Wk.T).reshape(b_, n, H, dk).transpose(0, 2, 1, 3)
    v = (x @ Wv.T).reshape(b_, n, H, dk).transpose(0, 2, 1, 3)
    s = np.einsum("bhqd,bhkd->bhqk", q, k) / np.sqrt(dk)
    mask = np.asarray(tgt_mask)[None, None] | \
        np.asarray(tgt_key_pad_mask)[:, None, None, :]
    s = np.where(mask, -np.inf, s)
    s = s - s.max(-1, keepdims=True)
    e = np.exp(s)
    att = e / e.sum(-1, keepdims=True)
    o = np.einsum("bhqk,bhkd->bhqd", att, v).transpose(0, 2, 1, 3).reshape(
        b_, n, d)
    h = o @ Wo.T + bo
    x1 = ln(x + h, g1, beta1)
    gl = x1 @ W1.T + b1
    gl = 0.5 * gl * (1 + np.tanh(np.sqrt(2 / np.pi) * (gl + 0.044715 * gl**3)))
    m = gl @ W2.T + b2
    return ln(x1 + m, g2, beta2).astype(np.float32)


def kernel(**inputs):
    in_maps = make_in_maps(**inputs)
    if in_maps is None:
        return _numpy_reference(**inputs)
    run = _get_runner()
    results = run(in_maps)
    out = np.empty((B, N, D), np.float32)
    for g in range(NCORES):
        b, r = g // TPR, g % TPR
        out[b, TS * r:TS * (r + 1), :] = results[g]["out"].T
    return out


# revision 41
# speedup vs baseline: 7561.0857x; 17.9279x over previous
"""Trainium2 Bass kernel for a causal self-attention transformer layer.

Layer (PostNorm, eval):
    h  = MHA_causal(tgt); x = LN(tgt + h); out = LN(x + gelu(x@W1.T+b1)@W2.T+b2)
Shapes: B=2, N=2048, D=1024, H=16 (dk=64), FFN=4096.

Distribution over 8 NeuronCores (core g: batch b=g//4, rank r=g%4):
  Stage A (head-parallel): core computes Q/K/V + causal attention for its 4
    heads over all 2048 tokens of its batch. Activations are kept
    feature-major [feat, tok] so every matmul contracts on partitions.
    Softmax uses no max-subtraction (scores are O(1) here); the denominator
    is produced for free as an extra all-ones column of V in the AV matmul.
  AllToAll (8 cores) converts head-sharded attention output into
    token-sharded (512 tokens/core). Cross-batch slots carry duplicate data
    which is nulled by zero-padded Wo weights.
  Stage B (token-parallel): Wo + bias + residual, LN1, W1+gelu, W2 + bias +
    residual, LN2 for the core's 512 tokens. LayerNorm over the feature
    (partition) axis uses ones-vector matmuls for the sums and K=1
    broadcast matmuls (float32r) to spread the per-token mean/rstd.

Host side shards/transposes/casts inputs, runs the SPMD program via a
cached PJRT callable, and reassembles the full [2, 2048, 1024] output.
"""

import numpy as np
import ml_dtypes

import concourse.bass as bass
import concourse.mybir as mybir
import concourse.tile as tile
from concourse.vector_clock import ScopedClock

BF16 = mybir.dt.bfloat16
F32 = mybir.dt.float32
F32R = mybir.dt.float32r
AF = mybir.ActivationFunctionType
ALU = mybir.AluOpType

B, N, D, H, DK, FFN = 2, 2048, 1024, 16, 64, 4096
EPS = 1e-5
NCORES = 8
TPR = 4            # tensor-parallel ranks per batch
HL = H // TPR      # heads per core (4)
DVL = HL * DK      # local head width (256)
TS = N // TPR      # tokens per core in stage B (512)
NP = 128           # partitions
QT = 512           # q tile width
NKB = N // NP      # key blocks (16)

# ---------------------------------------------------------------------------
# Walrus in this environment encodes at most ONE sync-wait per instruction.
# Patch Tile's exit drain and post-split every multi-wait instruction.
# ---------------------------------------------------------------------------

_wsplit = [0]


def _patched_drain_and_barrier(self, tick_clock, wait_clock):
    nc = self.nc
    probe = nc.sync.nop(nofuse=True)
    wait_clock.add_sem_waits(probe.ins, ScopedClock({None: tick_clock.global_clock}))
    si = probe.ins.sync_info
    waits = list(si.on_wait) if si is not None else []
    if waits:
        probe.ins.sync_info = mybir.SyncInfo(on_wait=[waits[0]], on_update=[])
        for w in waits[1:]:
            extra = nc.sync.nop(nofuse=True)
            extra.ins.sync_info = mybir.SyncInfo(on_wait=[w], on_update=[])
    nc.sync.drain()
    nc.all_engine_barrier()
    popped = nc._tile_sem_poison_stack.pop()
    assert popped is self._sem_poison
    nc.clear_and_free_semaphores(list(self.sems.allocated().values()))
    nc.all_engine_barrier()


tile.TileContext._drain_and_barrier = _patched_drain_and_barrier


def _split_multiwait_instructions(nc):
    for fn in nc.m.functions:
        for bb in fn.blocks:
            insts = bb.instructions
            if not any(
                i.sync_info is not None and len(i.sync_info.on_wait) > 1
                for i in insts
            ):
                continue
            new = []
            for inst in insts:
                si = inst.sync_info
                waits = list(si.on_wait) if si is not None else []
                if len(waits) > 1:
                    for w in waits[:-1]:
                        _wsplit[0] += 1
                        new.append(mybir.InstNoOp(
                            name=f"wsplit-{_wsplit[0]}",
                            engine=inst.engine,
                            sync_info=mybir.SyncInfo(on_wait=[w], on_update=[]),
                        ))
                    inst.sync_info = mybir.SyncInfo(
                        on_wait=[waits[-1]], on_update=list(si.on_update)
                    )
                new.append(inst)
            bb.instructions = new


# ---------------------------------------------------------------------------
# Program builder
# ---------------------------------------------------------------------------

def _build_program(for_sim=False, split=True):
    nc = bass.Bass("TRN2", target_bir_lowering=False, debug=False,
                   num_devices=NCORES)

    def din(name, shape, dt):
        return nc.dram_tensor(name, shape, dt, kind="ExternalInput").ap()

    xt_d = din("xt", [D, N], BF16)            # tgt[b].T
    xtr_d = din("xtr", [D, TS], F32)          # residual slice (my tokens)
    wqt_d = din("wqt", [D, DVL], BF16)        # Wq[local].T
    wkt_d = din("wkt", [D, DVL], BF16)
    wvt_d = din("wvt", [D, DVL], BF16)
    wot_d = din("wot", [2 * D, D], BF16)      # zero-padded Wo.T (A2A slots)
    w1t_d = din("w1t", [D, FFN], BF16)
    w2t_d = din("w2t", [FFN, D], BF16)
    bo_d = din("bo", [NP, D // NP], F32)
    b1_d = din("b1", [NP, FFN // NP], F32)
    b2_d = din("b2", [NP, D // NP], F32)
    mask_d = din("mask2", [NP, 4, 2 * QT], BF16)
    out_d = nc.dram_tensor("out", [D, TS], F32, kind="ExternalOutput").ap()

    HTS = TS // 2
    cc_in = [nc.dram_tensor(f"cc_in{i}", [NCORES * DVL, HTS], BF16).ap()
             for i in range(2)]
    cc_out = [nc.dram_tensor(f"cc_out{i}", [NCORES * DVL, HTS], BF16).ap()
              for i in range(2)]
    rb_d = nc.dram_tensor("rb_bounce", [4 * NCORES // 2 * 2, QT], F32).ap()

    with tile.TileContext(nc, num_cores=NCORES) as tc:
        with tc.tile_pool(name="const", bufs=1) as cpool:
            onesf = cpool.tile([NP, NP], F32)
            nc.vector.memset(onesf[:], 1.0)
            ones64r = cpool.tile([NP, 64], F32R)   # row 64 used by AV bcast
            nc.vector.tensor_copy(ones64r[64:65, :], onesf[64:65, 0:64])
            ones128r = cpool.tile([1, NP], F32R)   # LN bcast lhsT
            nc.vector.tensor_copy(ones128r[:], onesf[0:1, :])
            ones128b = cpool.tile([NP, 1], BF16)   # LN stats lhsT
            nc.vector.memset(ones128b[:], 1.0)
            onescol_r = cpool.tile([NP, 1], F32R)  # f32r variant
            nc.vector.tensor_copy(onescol_r[:], onesf[:, 0:1])
            bo_sb = cpool.tile([NP, D // NP], F32)
            nc.sync.dma_start(out=bo_sb[:], in_=bo_d[:])
            b1_sb = cpool.tile([NP, FFN // NP], F32)
            nc.sync.dma_start(out=b1_sb[:], in_=b1_d[:])
            b2_sb = cpool.tile([NP, D // NP], F32)
            nc.sync.dma_start(out=b2_sb[:], in_=b2_d[:])
            mask_sb = cpool.tile([NP, 4, 2 * QT], BF16)
            warm = cpool.tile([1, 16], F32)
            nc.scalar.activation(warm[:], onesf[0:1, 0:16], AF.Exp)

            # stage-B weight tiles allocated up front; their DMAs are
            # emitted after the attention-critical loads so they fill
            # spare DMA bandwidth during attention
            prefetch = tc.tile_pool(name="prefetch", bufs=1)
            pf = prefetch.__enter__()
            wot_sb = pf.tile([NP, 2 * D // NP, D], BF16)
            xtr_sb = pf.tile([NP, D // NP, TS], F32)

            # ---------------- Stage A: QKV + attention (4 local heads) ----
            with tc.tile_pool(name="sa", bufs=1) as sa:
                wk_sb = sa.tile([NP, D // NP, DVL], BF16)
                nc.sync.dma_start(
                    out=wk_sb[:], in_=wkt_d.rearrange("(c p) f -> p c f", p=NP))
                wq_sb = sa.tile([NP, D // NP, DVL], BF16)
                nc.sync.dma_start(
                    out=wq_sb[:], in_=wqt_d.rearrange("(c p) f -> p c f", p=NP))
                wv_sb = sa.tile([NP, D // NP, DVL], BF16)
                nc.sync.dma_start(
                    out=wv_sb[:], in_=wvt_d.rearrange("(c p) f -> p c f", p=NP))
                xt_c = []
                for dc in range(D // NP):
                    xc = sa.tile([NP, N], BF16, tag=f"xt{dc}", name=f"xt{dc}")
                    nc.sync.dma_start(
                        out=xc[:], in_=xt_d[dc * NP:(dc + 1) * NP, :])
                    xt_c.append(xc)

                q_sb = sa.tile([NP, 2, N], BF16)
                k_sb = sa.tile([NP, 2, N], BF16)
                v_sb = sa.tile([NP, NKB, HL * (DK + 1)], BF16)
                nc.sync.dma_start(out=mask_sb[:], in_=mask_d[:])
                nc.sync.dma_start(
                    out=wot_sb[:],
                    in_=wot_d.rearrange("(c p) f -> p c f", p=NP))
                nc.sync.dma_start(
                    out=xtr_sb[:],
                    in_=xtr_d.rearrange("(c p) t -> p c t", p=NP))

                attn_all = sa.tile([DK, HL * N], BF16)
                attn_h = [attn_all[:, h * N:(h + 1) * N] for h in range(HL)]

                def emit_k(ap, o, t):
                    ps2 = ap.tile([NP, QT], F32, tag="qkv", bufs=2,
                                  name=f"kps{o}_{t}")
                    for dc in range(D // NP):
                        nc.tensor.matmul(
                            ps2[:, 0:QT],
                            wk_sb[:, dc, o * NP:(o + 1) * NP],
                            xt_c[dc][:, t * QT:(t + 1) * QT],
                            start=(dc == 0), stop=(dc == D // NP - 1))
                    nc.vector.tensor_copy(
                        k_sb[:, o, t * QT:(t + 1) * QT], ps2[:, 0:QT])

                def emit_q(ap, o, t):
                    ps = ap.tile([NP, QT], F32, tag="qkv", bufs=2,
                                 name=f"qps{o}_{t}")
                    for dc in range(D // NP):
                        nc.tensor.matmul(
                            ps[:, 0:QT],
                            wq_sb[:, dc, o * NP:(o + 1) * NP],
                            xt_c[dc][:, t * QT:(t + 1) * QT],
                            start=(dc == 0), stop=(dc == D // NP - 1))
                    nc.vector.tensor_copy(
                        q_sb[:, o, t * QT:(t + 1) * QT], ps[:, 0:QT])

                def emit_v(ap, t):
                    psv = ap.tile([NP, QT], F32, tag="qkv", bufs=2,
                                  name=f"vps{t}")
                    for dc in range(D // NP):
                        nc.tensor.matmul(
                            psv[:, 0:DVL],
                            xt_c[dc][:, t * NP:(t + 1) * NP],
                            wv_sb[:, dc, :],
                            start=(dc == 0), stop=(dc == D // NP - 1))
                    vview = v_sb[:, t, :].rearrange("p (h c) -> p h c",
                                                    c=DK + 1)
                    nc.vector.tensor_copy(
                        vview[:, :, 0:DK],
                        psv[:, 0:DVL].rearrange("p (h c) -> p h c", c=DK))
                    nc.vector.memset(vview[:, :, DK:DK + 1], 1.0)

                with tc.tile_pool(name="att_ps", bufs=1, space="PSUM") as ap:
                    for j in range(N // QT):
                        # inputs this q-tile needs, emitted just-in-time so
                        # their PE work hides under the ACT-bound exp stream
                        for o in range(2):
                            emit_k(ap, o, j)
                        for o in range(2):
                            emit_q(ap, o, j)
                        for t in range(4 * j, 4 * (j + 1)):
                            emit_v(ap, t)
                        for hp in range(2):
                            h0, h1 = 2 * hp, 2 * hp + 1
                            pav0 = ap.tile([NP, QT], F32, tag="av0", bufs=1)
                            pav1 = ap.tile([NP, QT], F32, tag="av1", bufs=1)
                            nkb = (j + 1) * (QT // NP)
                            for kb in range(nkb):
                                ps_s = ap.tile([NP, 2 * QT], F32, tag="s",
                                               bufs=2)
                                nc.tensor.matmul(
                                    ps_s[:, 0:QT],
                                    k_sb[0:64, hp, kb * NP:(kb + 1) * NP],
                                    q_sb[0:64, hp, j * QT:(j + 1) * QT],
                                    start=True, stop=True)
                                nc.tensor.matmul(
                                    ps_s[:, QT:2 * QT],
                                    k_sb[64:NP, hp, kb * NP:(kb + 1) * NP],
                                    q_sb[64:NP, hp, j * QT:(j + 1) * QT],
                                    start=True, stop=True)
                                e_sb = sa.tile([NP, 2 * QT], BF16, tag="e",
                                               bufs=3)
                                nc.scalar.activation(
                                    e_sb[:], ps_s[:], AF.Exp,
                                    scale=1.0 / np.sqrt(DK))
                                di = kb - (QT // NP) * j
                                if di >= 0:
                                    nc.vector.tensor_tensor(
                                        e_sb[:], e_sb[:], mask_sb[:, di, :],
                                        op=ALU.mult)
                                nc.tensor.matmul(
                                    pav0[0:DK + 1, :],
                                    v_sb[:, kb, h0 * (DK + 1):(h0 + 1) * (DK + 1)],
                                    e_sb[:, 0:QT],
                                    start=(kb == 0), stop=(kb == nkb - 1))
                                nc.tensor.matmul(
                                    pav1[0:DK + 1, :],
                                    v_sb[:, kb, h1 * (DK + 1):(h1 + 1) * (DK + 1)],
                                    e_sb[:, QT:2 * QT],
                                    start=(kb == 0), stop=(kb == nkb - 1))
                            for e01, pav, h in ((0, pav0, h0), (1, pav1, h1)):
                                site = (j * 2 + hp) * 2 + e01
                                # drain the accumulator to SBUF right away
                                # so the PSUM bank frees for the next pair
                                # (GPSIMD cannot read PSUM; alternate the
                                # two least-loaded compute engines)
                                av_sb = sa.tile([DK + 1, QT], F32,
                                                tag="avsb", bufs=3)
                                if e01 == 0:
                                    nc.scalar.activation(
                                        av_sb[:], pav[0:DK + 1, :], AF.Copy)
                                else:
                                    nc.vector.tensor_copy(
                                        av_sb[:], pav[0:DK + 1, :])
                                rr = sa.tile([NP, QT], F32, tag="rr", bufs=2)
                                nc.vector.reciprocal(
                                    rr[64:65, :], av_sb[DK:DK + 1, :])
                                nc.sync.dma_start(
                                    out=rb_d[site:site + 1, :],
                                    in_=rr[64:65, :])
                                rbc = sa.tile([DK, QT], F32, tag="rbc", bufs=3)
                                nc.sync.dma_start(
                                    out=rbc[:],
                                    in_=rb_d[site:site + 1, :]
                                    .partition_broadcast(DK))
                                nc.vector.tensor_tensor(
                                    attn_h[h][:, j * QT:(j + 1) * QT],
                                    av_sb[0:DK, :], rbc[:], op=ALU.mult)
                        # stage chunk j of all 4 heads into cc_in slots
                        # j and j+4 of both halves
                        for h in range(HL):
                            for i in range(2):
                                src = attn_all[:, h * N + j * QT + i * HTS:
                                               h * N + j * QT + (i + 1) * HTS]
                                for s in (j, j + TPR):
                                    nc.sync.dma_start(
                                        out=cc_in[i][DVL * s + DK * h:
                                                     DVL * s + DK * (h + 1), :],
                                        in_=src)

            for i in range(2):
                if for_sim:
                    # stand-in for the A2A so the single-core timeline sim
                    # runs: same bytes through the DMA path
                    nc.sync.dma_start(out=cc_out[i][:], in_=cc_in[i][:])
                else:
                    nc.gpsimd.collective_compute(
                        "AllToAll", ALU.bypass,
                        ins=[cc_in[i][:]], outs=[cc_out[i][:]],
                        replica_groups=[list(range(NCORES))],
                    )

            # ---------------- Stage B: Wo + LN1 + MLP + LN2, two ------
            # 256-token halves pipelined against the two AllToAlls
            def _sel(x, ob):
                return x[ob][:] if isinstance(x, list) else x[:, ob, :]

            def layer_norm(tc, pool, src, outf, outb, W):
                """src [NP, 8, W] f32; outf/outb tensors or per-ob lists."""
                nblk = D // NP
                sbf = pool.tile([NP, nblk, W], BF16, tag="ln_bf")
                sqb = pool.tile([NP, nblk, W], BF16, tag="ln_sq")
                with tc.tile_pool(name="ln_ps", bufs=1, space="PSUM") as lp:
                    pmu = lp.tile([1, W], F32, tag="mu")
                    psq = lp.tile([1, W], F32, tag="sq")
                    for ob in range(nblk):
                        nc.vector.tensor_copy(sbf[:, ob, :], src[:, ob, :])
                        nc.vector.tensor_tensor(
                            sqb[:, ob, :], sbf[:, ob, :], sbf[:, ob, :],
                            op=ALU.mult)
                        nc.tensor.matmul(pmu[:], ones128b[:], sbf[:, ob, :],
                                         start=(ob == 0), stop=(ob == nblk - 1))
                        nc.tensor.matmul(psq[:], ones128b[:], sqb[:, ob, :],
                                         start=(ob == 0), stop=(ob == nblk - 1))
                    rows = pool.tile([1, 7, W], F32, tag="ln_rows")
                    mu, ex2, mu2, var = (rows[:, i, :] for i in range(4))
                    vr, rstd, brow = (rows[:, i, :] for i in range(4, 7))
                    nc.vector.tensor_scalar_mul(mu, pmu, 1.0 / D)
                    nc.vector.tensor_scalar_mul(ex2, psq[:], 1.0 / D)
                    nc.vector.tensor_tensor(mu2, mu, mu, op=ALU.mult)
                    nc.vector.tensor_tensor(var, ex2, mu2, op=ALU.subtract)
                    nc.vector.tensor_scalar_add(var, var, EPS)
                    nc.vector.reciprocal(vr, var)
                    nc.scalar.activation(rstd, vr, AF.Sqrt)
                    nc.vector.tensor_tensor(brow, mu, rstd, op=ALU.mult)
                    rowr = pool.tile([1, 2, W], F32R, tag="ln_rowr")
                    nc.vector.tensor_copy(rowr[:, 0, :], rstd)
                    nc.vector.tensor_scalar_mul(rowr[:, 1, :], brow, -1.0)
                    pA = lp.tile([NP, W], F32, tag="bA")
                    pB = lp.tile([NP, W], F32, tag="bB")
                    pA, pB = pA[:], pB[:]
                    nc.tensor.matmul(pA, ones128r[:], rowr[:, 0, :],
                                     start=True, stop=True)
                    nc.tensor.matmul(pB, ones128r[:], rowr[:, 1, :],
                                     start=True, stop=True)
                    A_sb = pool.tile([NP, W], F32, tag="ln_A")
                    B_sb = pool.tile([NP, W], F32, tag="ln_B")
                    nc.scalar.activation(A_sb[:], pA, AF.Copy)
                    nc.scalar.activation(B_sb[:], pB, AF.Copy)
                    if outb is not None:
                        # bf16 fast path first so downstream matmuls unblock
                        Ab = pool.tile([NP, W], BF16, tag="ln_Ab")
                        Bb = pool.tile([NP, W], BF16, tag="ln_Bb")
                        nc.scalar.activation(Ab[:], pA, AF.Copy)
                        nc.scalar.activation(Bb[:], pB, AF.Copy)
                        tmpb = pool.tile([NP, nblk, W], BF16, tag="ln_tmpb")
                        for ob in range(nblk):
                            nc.vector.tensor_tensor(
                                tmpb[:, ob, :], _sel(src, ob), Ab[:],
                                op=ALU.mult)
                            nc.vector.tensor_tensor(
                                _sel(outb, ob), tmpb[:, ob, :], Bb[:],
                                op=ALU.add)
                    tmp = pool.tile([NP, nblk, W], F32, tag="ln_tmp")
                    for ob in range(nblk):
                        nc.vector.tensor_tensor(
                            tmp[:, ob, :], src[:, ob, :], A_sb[:], op=ALU.mult)
                        nc.vector.tensor_tensor(
                            _sel(outf, ob), tmp[:, ob, :], B_sb[:], op=ALU.add)

            with tc.tile_pool(name="sbw", bufs=1) as sbw:
                x2f = [sbw.tile([NP, TS], F32, tag=f"x2f{ob}",
                                name=f"x2f{ob}") for ob in range(D // NP)]
                x2b = [sbw.tile([NP, TS], BF16, tag=f"x2b{ob}",
                                name=f"x2b{ob}") for ob in range(D // NP)]

                def wo_half(h01, wp, sbo):
                    hsl = slice(h01 * HTS, (h01 + 1) * HTS)
                    ao = sbo.tile([NP, 2 * D // NP, HTS], BF16, tag="ao",
                                  name=f"ao_{h01}")
                    for c in range(2 * D // NP):
                        nc.sync.dma_start(
                            out=ao[:, c, :],
                            in_=cc_out[h01][c * NP:(c + 1) * NP, :])
                    sum1 = sbo.tile([NP, D // NP, HTS], F32R, tag="sum1",
                                    name=f"sum1_{h01}")
                    for ob in range(D // NP):
                        ph = wp.tile([NP, HTS], F32, tag=f"m{h01}", bufs=2)
                        for c in range(2 * D // NP):
                            nc.tensor.matmul(
                                ph[:], wot_sb[:, c, ob * NP:(ob + 1) * NP],
                                ao[:, c, :],
                                start=(c == 0), stop=(c == 2 * D // NP - 1))
                        hb = sbo.tile([NP, HTS], F32, tag=f"hb{h01}", bufs=2)
                        nc.scalar.activation(hb[:], ph[:], AF.Identity,
                                             bias=bo_sb[:, ob:ob + 1])
                        nc.vector.tensor_tensor(
                            sum1[:, ob, :], hb[:], xtr_sb[:, ob, hsl],
                            op=ALU.add)
                    return sum1

                def ln_sq(pool, src, W, pre=None):
                    nblk = D // NP
                    sqb = pool.tile([NP, nblk, W], F32R, tag="ln_sq")
                    for ob in range(nblk):
                        if pre is not None:
                            pre(ob)
                        nc.vector.tensor_tensor(
                            sqb[:, ob, :], src[:, ob, :], src[:, ob, :],
                            op=ALU.mult)
                    return sqb

                def ln_rows(pool, lp, src, sqb, W):
                    nblk = D // NP
                    pmu = lp.tile([1, W], F32, tag="statmu")
                    psq = lp.tile([1, W], F32, tag="statsq")
                    pmu, psq = pmu[:], psq[:]
                    for ob in range(nblk):
                        nc.tensor.matmul(pmu, onescol_r[:], src[:, ob, :],
                                         start=(ob == 0), stop=(ob == nblk - 1))
                        nc.tensor.matmul(psq, onescol_r[:], sqb[:, ob, :],
                                         start=(ob == 0), stop=(ob == nblk - 1))
                    rows = pool.tile([1, 4, W], F32, tag="ln_rows")
                    mu, ex2e, mu2, vr = (rows[:, i, :] for i in range(4))
                    nc.vector.tensor_scalar_mul(mu, pmu, 1.0 / D)
                    # ex2e = psq/D + EPS in one fused op
                    nc.vector.tensor_scalar(ex2e, psq, 1.0 / D, EPS,
                                            op0=ALU.mult, op1=ALU.add)
                    nc.vector.tensor_tensor(mu2, mu, mu, op=ALU.mult)
                    nc.vector.tensor_tensor(mu2, ex2e, mu2, op=ALU.subtract)
                    nc.vector.reciprocal(vr, mu2)
                    rowr = pool.tile([1, 2, W], F32R, tag="ln_rowr")
                    nc.scalar.activation(rowr[:, 0, :], vr, AF.Sqrt)
                    # rowr1 = (mu * -1) * rstd in one fused op
                    nc.vector.scalar_tensor_tensor(
                        rowr[:, 1, :], mu, -1.0, rowr[:, 0, :],
                        op0=ALU.mult, op1=ALU.mult)
                    return None, rowr

                def ln_finish(pool, lp, src, sbf, rowr, outf, outb, W):
                    nblk = D // NP
                    pA = lp.tile([NP, W], F32, tag="bA")
                    pB = lp.tile([NP, W], F32, tag="bB")
                    pA, pB = pA[:], pB[:]
                    nc.tensor.matmul(pA, ones128r[:], rowr[:, 0, :],
                                     start=True, stop=True)
                    nc.tensor.matmul(pB, ones128r[:], rowr[:, 1, :],
                                     start=True, stop=True)
                    A_sb = pool.tile([NP, W], F32, tag="ln_A")
                    B_sb = pool.tile([NP, W], F32, tag="ln_B")
                    nc.scalar.activation(A_sb[:], pA, AF.Copy)
                    nc.scalar.activation(B_sb[:], pB, AF.Copy)
                    if outb is not None:
                        Ab = pool.tile([NP, W], BF16, tag="ln_Ab")
                        Bb = pool.tile([NP, W], BF16, tag="ln_Bb")
                        nc.scalar.activation(Ab[:], pA, AF.Copy)
                        nc.scalar.activation(Bb[:], pB, AF.Copy)
                        tmpb = pool.tile([NP, nblk, W], BF16, tag="ln_tmpb")
                        for ob in range(nblk):
                            eng = nc.gpsimd if ob % 3 == 2 else nc.vector
                            eng.tensor_tensor(
                                tmpb[:, ob, :], _sel(src, ob), Ab[:],
                                op=ALU.mult)
                            nc.vector.tensor_tensor(
                                outb[ob], tmpb[:, ob, :], Bb[:], op=ALU.add)

                    def fp32_path():
                        tmp = pool.tile([NP, nblk, W], F32, tag="ln_tmp")
                        for ob in range(nblk):
                            eng = nc.gpsimd if (outb is None and ob >= 5) \
                                else nc.vector
                            eng.tensor_tensor(
                                tmp[:, ob, :], _sel(src, ob), A_sb[:],
                                op=ALU.mult)
                            nc.vector.tensor_tensor(
                                outf[ob], tmp[:, ob, :], B_sb[:], op=ALU.add)
                    return fp32_path

                def _sel(x, ob):
                    return x[ob][:] if isinstance(x, list) else x[:, ob, :]

                hs = [slice(0, HTS), slice(HTS, TS)]
                g_all = sbw.tile([NP, FFN // NP, TS], BF16)
                with tc.tile_pool(name="sbo", bufs=1) as sbo, \
                     tc.tile_pool(name="ln1p", bufs=1) as lnp, \
                     tc.tile_pool(name="ln1_ps", bufs=1, space="PSUM") as lp, \
                     tc.tile_pool(name="w1s", bufs=1) as w1s, \
                     tc.tile_pool(name="w1_ps", bufs=1, space="PSUM") as mp:
                    wp = mp
                    # interleaved emission: PE stays busy on the other half
                    # while DVE/ACT run each half's LN row chain
                    sum1_0 = wo_half(0, wp, sbo)
                    sum1_1 = wo_half(1, wp, sbo)
                    sq0 = ln_sq(lnp, sum1_0, HTS)
                    sbf0, rowr0 = ln_rows(lnp, lp, sum1_0, sq0, HTS)
                    fp0 = ln_finish(lnp, lp, sum1_0, sbf0, rowr0,
                                    [x[:, hs[0]] for x in x2f],
                                    [x[:, hs[0]] for x in x2b], HTS)
                    sq1 = ln_sq(lnp, sum1_1, HTS)
                    sbf1, rowr1 = ln_rows(lnp, lp, sum1_1, sq1, HTS)
                    fp1 = ln_finish(lnp, lp, sum1_1, sbf1, rowr1,
                                    [x[:, hs[1]] for x in x2f],
                                    [x[:, hs[1]] for x in x2b], HTS)

                    # W1 + gelu: single weight stream; half-1 lags 2 chunks
                    w1cs = {}
                    LAG = 2
                    def w1_group(fc, h01):
                        w1c = w1cs[fc]
                        for fs in range(QT // NP):
                            fb = fc * (QT // NP) + fs
                            pm = mp.tile([NP, HTS], F32, tag=f"m{h01}",
                                         bufs=2)
                            for dc in range(D // NP):
                                nc.tensor.matmul(
                                    pm[:], w1c[:, dc, fs * NP:(fs + 1) * NP],
                                    x2b[dc][:, hs[h01]],
                                    start=(dc == 0), stop=(dc == D // NP - 1))
                            nc.scalar.activation(
                                g_all[:, fb, hs[h01]], pm[:],
                                AF.Sigmoid if for_sim else AF.Gelu_apprx_tanh,
                                bias=b1_sb[:, fb:fb + 1])
                    for fc in range(FFN // QT + LAG):
                        if fc == 1:
                            # x2f fp32 chains run on DVE while W1 streams
                            fp0()
                            fp1()
                        if fc < FFN // QT:
                            w1c = w1s.tile([NP, D // NP, QT], BF16, tag="w1c",
                                           bufs=LAG + 1, name=f"w1c{fc}")
                            nc.sync.dma_start(
                                out=w1c[:],
                                in_=w1t_d[:, fc * QT:(fc + 1) * QT].rearrange(
                                    "(c p) f -> p c f", p=NP))
                            w1cs[fc] = w1c
                            w1_group(fc, 0)
                        if fc >= LAG:
                            w1_group(fc - LAG, 1)
                            del w1cs[fc - LAG]

                # W2 full-width: one weight stream, 8 psum accumulators
                with tc.tile_pool(name="w2s", bufs=1) as w2s:
                    sum2 = w2s.tile([NP, D // NP, TS], F32R)
                    yp_cm = tc.tile_pool(name="w2_ps", bufs=1, space="PSUM")
                    yp = yp_cm.__enter__()
                    pys = [yp.tile([NP, TS], F32, tag=f"y{ob}", name=f"y{ob}")
                           for ob in range(D // NP)]
                    for fb in range(FFN // NP):
                        w2c = w2s.tile([NP, D], BF16, tag="w2c", bufs=3)
                        nc.sync.dma_start(
                            out=w2c[:], in_=w2t_d[fb * NP:(fb + 1) * NP, :])
                        for ob in range(D // NP):
                            nc.tensor.matmul(
                                pys[ob][:], w2c[:, ob * NP:(ob + 1) * NP],
                                g_all[:, fb, :],
                                start=(fb == 0), stop=(fb == FFN // NP - 1))
                    def w2_tail(ob):
                        mb = w2s.tile([NP, TS], F32, tag="mb", bufs=2,
                                      name=f"mb{ob}")
                        nc.scalar.activation(mb[:], pys[ob][:], AF.Identity,
                                             bias=b2_sb[:, ob:ob + 1])
                        nc.vector.tensor_tensor(
                            sum2[:, ob, :], mb[:], x2f[ob][:], op=ALU.add)

                    # LN2 full width, bias/residual fused into the sq loop
                    yf = w2s.tile([NP, D // NP, TS], F32)
                    with tc.tile_pool(name="ln2", bufs=1) as lnp2:
                        sq2 = ln_sq(lnp2, sum2, TS, pre=w2_tail)
                        yp_cm.__exit__(None, None, None)
                        with tc.tile_pool(name="ln2_ps", bufs=1,
                                          space="PSUM") as lp2:
                            fp2 = ln_finish(lnp2, lp2, sum2,
                                            *ln_rows(lnp2, lp2, sum2, sq2,
                                                     TS),
                                            [yf[:, ob, :]
                                             for ob in range(D // NP)],
                                            None, TS)
                            fp2()
                    for ob in range(D // NP):
                        nc.sync.dma_start(
                            out=out_d[ob * NP:(ob + 1) * NP, :],
                            in_=yf[:, ob, :])
            prefetch.__exit__(None, None, None)

    if split:
        _split_multiwait_instructions(nc)
    return nc


# ---------------------------------------------------------------------------
# Cached PJRT runner (mirrors bass2jax.run_bass_via_pjrt multi-core path but
# keeps the jitted callable so repeat calls don't recompile).
# ---------------------------------------------------------------------------

_RUNNER = None


def _make_runner(nc):
    import jax
    from jax.sharding import Mesh, PartitionSpec
    from jax.experimental.shard_map import shard_map
    from concourse import bass2jax

    bass2jax.install_neuronx_cc_hook()
    partition_name = (nc.partition_id_tensor.name
                      if nc.partition_id_tensor else None)
    in_names, out_names, out_avals = [], [], []
    for alloc in nc.m.functions[0].allocations:
        if not isinstance(alloc, mybir.MemoryLocationSet):
            continue
        name = alloc.memorylocations[0].name
        if alloc.kind == "ExternalInput":
            if name != partition_name:
                in_names.append(name)
        elif alloc.kind == "ExternalOutput":
            out_names.append(name)
            out_avals.append(jax.core.ShapedArray(
                tuple(alloc.tensor_shape), mybir.dt.np(alloc.dtype)))
    n_params = len(in_names)
    all_in_names = list(in_names) + list(out_names)
    if partition_name is not None:
        all_in_names.append(partition_name)
    donate = tuple(range(n_params, n_params + len(out_names)))

    def _body(*args):
        operands = list(args)
        if partition_name is not None:
            operands.append(bass2jax.partition_id_tensor())
        outs = bass2jax._bass_exec_p.bind(
            *operands,
            out_avals=tuple(out_avals),
            in_names=tuple(all_in_names),
            out_names=tuple(out_names),
            lowering_input_output_aliases=(),
            sim_require_finite=True,
            sim_require_nnan=True,
            nc=nc,
        )
        return tuple(outs)

    devices = jax.devices()[:NCORES]
    mesh = Mesh(np.asarray(devices), ("core",))
    specs = (PartitionSpec("core"),) * (n_params + len(out_names))
    sharded = jax.jit(
        shard_map(_body, mesh=mesh, in_specs=specs,
                  out_specs=(PartitionSpec("core"),) * len(out_names),
                  check_rep=False),
        donate_argnums=donate, keep_unused=True)

    from jax.sharding import NamedSharding
    shard = NamedSharding(mesh, PartitionSpec("core"))

    def prepare(in_maps):
        per_core = [[np.asarray(m[name]) for name in in_names]
                    for m in in_maps]
        concat_in = [np.concatenate([per_core[c][i] for c in range(NCORES)],
                                    axis=0) for i in range(n_params)]
        return [jax.device_put(a, shard) for a in concat_in]

    def run_prepared(dev_in, materialize=True):
        concat_zeros = [np.zeros((NCORES * a.shape[0], *a.shape[1:]), a.dtype)
                        for a in out_avals]
        out_arrs = sharded(*dev_in, *concat_zeros)
        if not materialize:
            jax.block_until_ready(out_arrs)
            return None
        return [
            {name: np.asarray(out_arrs[i]).reshape(
                NCORES, *out_avals[i].shape)[c]
             for i, name in enumerate(out_names)}
            for c in range(NCORES)
        ]

    def run_prepared_async(dev_in):
        concat_zeros = [np.zeros((NCORES * a.shape[0], *a.shape[1:]), a.dtype)
                        for a in out_avals]
        return sharded(*dev_in, *concat_zeros)

    def run(in_maps, materialize=True):
        return run_prepared(prepare(in_maps), materialize)

    run.prepare = prepare
    run.run_prepared = run_prepared
    run.run_prepared_async = run_prepared_async
    return run


def _get_runner():
    global _RUNNER
    if _RUNNER is None:
        nc = _build_program()
        _RUNNER = _make_runner(nc)
    return _RUNNER


# ---------------------------------------------------------------------------
# Host-side sharding / gathering
# ---------------------------------------------------------------------------

def _bf16(a):
    return np.ascontiguousarray(a).astype(ml_dtypes.bfloat16)


def make_in_maps(tgt, tgt_mask, tgt_key_pad_mask, Wq, Wk, Wv, Wo, bo,
                 W1, b1, W2, b2, g1, beta1, g2, beta2):
    causal = np.triu(np.ones((N, N), bool), k=1)
    if not (np.array_equal(np.asarray(tgt_mask), causal)
            and not np.asarray(tgt_key_pad_mask).any()
            and np.allclose(np.asarray(g1), 1) and np.allclose(np.asarray(g2), 1)
            and np.allclose(np.asarray(beta1), 0)
            and np.allclose(np.asarray(beta2), 0)):
        return None  # unsupported masking/affine -> numpy fallback

    tgt = np.asarray(tgt, np.float32)
    Wq, Wk, Wv, Wo = (np.asarray(a, np.float32) for a in (Wq, Wk, Wv, Wo))
    W1, W2 = np.asarray(W1, np.float32), np.asarray(W2, np.float32)
    bo, b1, b2 = (np.asarray(a, np.float32) for a in (bo, b1, b2))

    # diagonal-block multiplicative masks [128, 4, 2*QT]
    q_idx = np.arange(QT)[None, :]
    mask2 = np.empty((NP, 4, 2 * QT), np.float32)
    for i in range(4):
        m = (NP * i + np.arange(NP)[:, None] <= q_idx).astype(np.float32)
        mask2[:, i, :] = np.concatenate([m, m], axis=1)

    w1t = _bf16(W1.T)
    w2t = _bf16(W2.T)
    bo_c = np.ascontiguousarray(bo.reshape(D // NP, NP).T)
    b1_c = np.ascontiguousarray(b1.reshape(FFN // NP, NP).T)
    b2_c = np.ascontiguousarray(b2.reshape(D // NP, NP).T)
    mask2_bf = _bf16(mask2)

    xt_b = [_bf16(tgt[b].T) for b in range(B)]
    in_maps = []
    for g in range(NCORES):
        b, r = g // TPR, g % TPR
        sl = slice(DVL * r, DVL * (r + 1))
        wot_ext = np.zeros((2 * D, D), np.float32)
        wot_ext[D * b:D * (b + 1), :] = Wo.T
        in_maps.append({
            "xt": xt_b[b],
            "xtr": np.ascontiguousarray(tgt[b].T[:, TS * r:TS * (r + 1)]),
            "wqt": _bf16(Wq[sl, :].T),
            "wkt": _bf16(Wk[sl, :].T),
            "wvt": _bf16(Wv[sl, :].T),
            "wot": _bf16(wot_ext),
            "w1t": w1t,
            "w2t": w2t,
            "bo": bo_c,
            "b1": b1_c,
            "b2": b2_c,
            "mask2": mask2_bf,
        })
    return in_maps


def _numpy_reference(tgt, tgt_mask, tgt_key_pad_mask, Wq, Wk, Wv, Wo, bo,
                     W1, b1, W2, b2, g1, beta1, g2, beta2):
    def ln(x, g, b):
        mu = x.mean(-1, keepdims=True)
        var = ((x - mu) ** 2).mean(-1, keepdims=True)
        return (x - mu) / np.sqrt(var + EPS) * g + b

    x = np.asarray(tgt, np.float64)
    b_, n, d = x.shape
    dk = d // H
    q = (x @ Wq.T).reshape(b_, n, H, dk).transpose(0, 2, 1, 3)
    k = (x @ Wk.T).reshape(b_, n, H, dk).transpose(0, 2, 1, 3)
    v = (x @ Wv.T).reshape(b_, n, H, dk).transpose(0, 2, 1, 3)
    s = np.einsum("bhqd,bhkd->bhqk", q, k) / np.sqrt(dk)
    mask = np.asarray(tgt_mask)[None, None] | \
        np.asarray(tgt_key_pad_mask)[:, None, None, :]
    s = np.where(mask, -np.inf, s)
    s = s - s.max(-1, keepdims=True)
    e = np.exp(s)
    att = e / e.sum(-1, keepdims=True)
    o = np.einsum("bhqk,bhkd->bhqd", att, v).transpose(0, 2, 1, 3).reshape(
        b_, n, d)
    h = o @ Wo.T + bo
    x1 = ln(x + h, g1, beta1)
    gl = x1 @ W1.T + b1
    gl = 0.5 * gl * (1 + np.tanh(np.sqrt(2 / np.pi) * (gl + 0.044715 * gl**3)))
    m = gl @ W2.T + b2
    return ln(x1 + m, g2, beta2).astype(np.float32)


def kernel(**inputs):
    in_maps = make_in_maps(**inputs)
    if in_maps is None:
        return _numpy_reference(**inputs)
    run = _get_runner()
    results = run(in_maps)
    out = np.empty((B, N, D), np.float32)
    for g in range(NCORES):
        b, r = g // TPR, g % TPR
        out[b, TS * r:TS * (r + 1), :] = results[g]["out"].T
    return out
